# revision 29
# baseline (speedup 1.0000x reference)
"""Trainium2 Bass kernel for a video-diffusion BasicTransformerBlock
(sparse-causal self-attn + cross-attn + GEGLU FF).

Sharding: data-parallel, one (batch, frame) pair per NeuronCore (8 frames ->
8 cores). Each core receives its own frame, frame 0 of its batch, and the
previous frame (duplicated inputs), so the sparse-causal KV gather needs no
collectives. For frames 0/1 the first/former KV frames coincide; softmax over
duplicated keys is mathematically identical to the reference's concat.

On-device layout: activations are feature-major (x^T, [dim, tokens]) so every
projection contracts over SBUF partitions without any transposes. LayerNorm
column-stats come from ones-matmuls; softmax runs max-free (scores are
bounded ~|5.5|) with denominators from an appended ones-column in V.
All transposes happen host-side in numpy.
"""
import os
import sys
import numpy as np

if not os.environ.get("TRN_TERMINAL_POOL_IPS"):
    raise RuntimeError("expected axon trn environment")
for _p in ("/opt/trn_rl_repo",):
    if _p not in sys.path:
        sys.path.append(_p)

import ml_dtypes
import concourse.bass as bass
import concourse.tile as tile
from concourse import bacc, mybir
from concourse.bass_utils import run_bass_kernel_spmd

FP32 = mybir.dt.float32
F32R = mybir.dt.float32r
BF16 = mybir.dt.bfloat16
AF = mybir.ActivationFunctionType
OP = mybir.AluOpType

D = 640          # model dim
T = 1024         # tokens / frame
H = 8            # heads
DH = 80          # head dim
DKT = D // 128   # 5 feature tiles of the model dim
TT = T // 128    # 8 token tiles / frame
QH = 512         # query half width
CROSS = 768
CKT = CROSS // 128
CTX = 77
CTXP = 80   # context padded for fp32r free-dim alignment
DFF = 2560       # ff hidden (per GEGLU half)
FMT = DFF // 128  # 20 ff row tiles per half
LN_EPS = 1e-5

# bias-pack column offsets ([128, NB] f32)
OB1, OB2, FB2, FBX, FBG = 0, 5, 10, 15, 35
LN_G = {1: 55, 2: 65, 3: 75}
LN_B = {1: 60, 2: 70, 3: 80}
EPS_COL = 85
NB = 86

N_CORES = 8

# test hook: CoreSim lacks Gelu; tests may override with a sim-supported func
GELU_AF = None


def r32(ap):
    return ap if ap.dtype == F32R else ap.bitcast(F32R)


def build_program(ln_trivial):
    nc = bacc.Bacc("TRN2", target_bir_lowering=False, debug=False,
                   num_devices=N_CORES)
    dram = {}
    for name in ("hsT_q", "hsT_first", "hsT_former"):
        dram[name] = nc.dram_tensor(name, [D, T], F32R, kind="ExternalInput").ap()
    dram["encT"] = nc.dram_tensor("encT", [CROSS, CTXP], F32R, kind="ExternalInput").ap()
    for name in ("q1", "k1", "v1", "q2"):
        dram[name] = nc.dram_tensor(name, [D, D], F32R, kind="ExternalInput").ap()
    for name in ("k2", "v2"):
        dram[name] = nc.dram_tensor(name, [CROSS, D], F32R, kind="ExternalInput").ap()
    for name in ("o1p", "o2p"):
        dram[name] = nc.dram_tensor(name, [H * 128, D], BF16, kind="ExternalInput").ap()
    dram["ff1b"] = nc.dram_tensor("ff1b", [2 * FMT, D, 128], F32R, kind="ExternalInput").ap()
    dram["ff2"] = nc.dram_tensor("ff2", [DFF, D], BF16, kind="ExternalInput").ap()
    dram["biases"] = nc.dram_tensor("biases", [128, NB], FP32, kind="ExternalInput").ap()
    out_dram = nc.dram_tensor("outT", [D, T], F32R, kind="ExternalOutput").ap()

    scale = float(DH) ** -0.5

    with tile.TileContext(nc) as tc:
        from contextlib import ExitStack
        with ExitStack() as ctx:
            pc = ctx.enter_context(tc.tile_pool(name="const", bufs=1))
            pres = ctx.enter_context(tc.tile_pool(name="res", bufs=5))
            pn = ctx.enter_context(tc.tile_pool(name="n", bufs=5))
            psq = ctx.enter_context(tc.tile_pool(name="sq", bufs=1))
            prow = ctx.enter_context(tc.tile_pool(name="row", bufs=1))
            pbc = ctx.enter_context(tc.tile_pool(name="bc", bufs=2))
            prcb = ctx.enter_context(tc.tile_pool(name="rcb", bufs=2))
            pw = ctx.enter_context(tc.tile_pool(name="w", bufs=6))
            pps = ctx.enter_context(tc.tile_pool(name="ps", bufs=2, space="PSUM"))

            bias_sb = pc.tile([128, NB], FP32, tag="bias")
            nc.sync.dma_start(bias_sb[:], dram["biases"][:])
            invd_f = pc.tile([128, 1], FP32, tag="invdf")
            nc.vector.memset(invd_f[:], 1.0 / D)
            invd = pc.tile([128, 1], F32R, tag="invd")
            nc.vector.tensor_copy(invd[:], invd_f[:])  # fp32r rounding producer
            onesr_f = pc.tile([128, 128], FP32, tag="onesrf")
            nc.vector.memset(onesr_f[:], 1.0)
            onesr = pc.tile([128, 128], F32R, tag="onesr")
            nc.vector.tensor_copy(onesr[:], onesr_f[:])

            def bcol(j):
                return bias_sb[:, j:j + 1]

            def load_w(dname, n_kt, tag, pool, dtype=F32R):
                tiles = []
                for kt in range(n_kt):
                    wt = pool.tile([128, D], dtype, tag=tag, name=f"{dname}_{kt}")
                    nc.sync.dma_start(wt[:], dram[dname][kt * 128:(kt + 1) * 128, :])
                    tiles.append(wt)
                return tiles

            def emit_ln(x_tiles, which, out_tiles):
                """Feature-major LN of 5 [128, T] fp32r tiles.

                Column stats via fp32r ones-matmuls; mean/rstd rows for the
                two query halves are packed at partitions 0/32 so one batched
                DVE reciprocal serves both, and broadcasting across
                partitions is a PE ones-column outer product into PSUM
                (gpsimd partition_broadcast corrupts offset-row sources on
                HW). out_tiles: list that receives the 5 result APs; passing
                x_tiles itself runs the LN in place."""
                in_place = out_tiles is x_tiles
                mup = prow.tile([128, QH], F32R, tag="mup", bufs=2, name=f"mup{which}")
                msqp = prow.tile([128, QH], FP32, tag="msqp", bufs=2, name=f"msqp{which}")
                rstd = prow.tile([128, QH], F32R, tag="rstd", bufs=2, name=f"rstd{which}")
                mu_b = {}
                for hh in range(2):
                    sl = slice(hh * QH, (hh + 1) * QH)
                    r0 = 32 * hh
                    sp = pps.tile([128, QH], FP32, tag="ps", name=f"lnps{which}{hh}")
                    spq = pps.tile([128, QH], FP32, tag="ps", name=f"lnpsq{which}{hh}")
                    for kt in range(DKT):
                        nc.tensor.matmul(sp[0:1, :], invd[:, 0:1],
                                         x_tiles[kt][:, sl],
                                         start=(kt == 0), stop=(kt == DKT - 1))
                    for kt in range(DKT):
                        sq = psq.tile([128, QH], F32R, tag="sq", name=f"sq{which}{hh}{kt}")
                        nc.scalar.square(sq[:], x_tiles[kt][:, sl])
                        nc.tensor.matmul(spq[0:1, :], invd[:, 0:1], sq[:],
                                         start=(kt == 0), stop=(kt == DKT - 1))
                    nc.vector.tensor_copy(mup[r0:r0 + 1, :], sp[0:1, :])
                    nc.vector.tensor_copy(msqp[r0:r0 + 1, :], spq[0:1, :])
                    mb = pps.tile([128, QH], FP32, tag="avps", bufs=2,
                                  name=f"mub{which}{hh}")
                    nc.tensor.matmul(mb[:, :], onesr[r0:r0 + 1, :],
                                     mup[r0:r0 + 1, :], start=True, stop=True)
                    mu_b[hh] = mb
                    # pass 1: x - mu (frees the mu broadcast PSUM bank early)
                    for kt in range(DKT):
                        if in_place:
                            nt_seg = x_tiles[kt][:, sl]
                        else:
                            if hh == 0:
                                nt = pn.tile([128, T], F32R, tag="n",
                                             name=f"n{which}_{kt}")
                                out_tiles.append(nt)
                            nt_seg = out_tiles[kt][:, sl]
                        nc.vector.tensor_tensor(nt_seg, x_tiles[kt][:, sl],
                                                mu_b[hh][:, :], OP.subtract)
                    # -var = mu^2 - E[x^2] at the packed row
                    nc.vector.tensor_tensor(mup[r0:r0 + 1, :], mup[r0:r0 + 1, :],
                                            mup[r0:r0 + 1, :], OP.mult)
                    nc.vector.tensor_tensor(mup[r0:r0 + 1, :], mup[r0:r0 + 1, :],
                                            msqp[r0:r0 + 1, :], OP.subtract)
                    # rstd = exp(-0.5 * ln(var + eps)); ACT Ln/Exp round trip
                    # measured at 1.1e-5 max rel on HW, and keeps the whole
                    # tail off the (busier) vector engine
                    nc.scalar.activation(msqp[r0:r0 + 1, :], mup[r0:r0 + 1, :],
                                         AF.Ln, scale=-1.0,
                                         bias=bias_sb[0:1, EPS_COL:EPS_COL + 1])
                    nc.scalar.activation(rstd[r0:r0 + 1, :], msqp[r0:r0 + 1, :],
                                         AF.Exp, scale=-0.5)
                for hh in range(2):
                    sl = slice(hh * QH, (hh + 1) * QH)
                    r0 = 32 * hh
                    rb = pps.tile([128, QH], FP32, tag="avps", bufs=2,
                                  name=f"rb{which}{hh}")
                    nc.tensor.matmul(rb[:, :], onesr[r0:r0 + 1, :],
                                     rstd[r0:r0 + 1, :], start=True, stop=True)
                    for kt in range(DKT):
                        nt_seg = (x_tiles[kt] if in_place else out_tiles[kt])[:, sl]
                        nc.vector.tensor_tensor(nt_seg, nt_seg, rb[:, :], OP.mult)
                        if not ln_trivial[which - 1]:
                            nc.scalar.activation(nt_seg, nt_seg, AF.Identity,
                                                 bias=bcol(LN_B[which] + kt),
                                                 scale=bcol(LN_G[which] + kt))
                return out_tiles

            def head_proj(w_tiles, n_tiles, out_tiles, col_off, n_kt, tag):
                """out^T[h][0:80, col_off:col_off+T] = w.T @ n, per-head padded."""
                for h in range(H):
                    for hh in range(2):
                        qp = pps.tile([128, QH], FP32, tag="ps", name=f"hp{tag}{h}{hh}")
                        for kt in range(n_kt):
                            nc.tensor.matmul(
                                qp[0:DH, :],
                                r32(w_tiles[kt][:, h * DH:(h + 1) * DH]),
                                r32(n_tiles[kt][:, hh * QH:(hh + 1) * QH]),
                                start=(kt == 0), stop=(kt == n_kt - 1))
                        nc.vector.tensor_copy(
                            out_tiles[h][0:DH, col_off + hh * QH:col_off + (hh + 1) * QH],
                            qp[0:DH, :])

            def v_proj(n_tiles, vt, n_kt, w_tiles, n_tok, tok_off):
                """token-major V tile, per-head 97-col slots: data cols 0:80,
                ones col at 96 so the AV denominator lands on PSUM partition
                96 (engine APs must start at partition 0/32/64/96)."""
                pad_ap = vt[:, 0:776].rearrange("p (h c) -> p h c", c=97)[:, :, 80:96]
                nc.vector.memset(pad_ap, 0.0)
                ones_ap = vt[:, 0:776].rearrange("p (h c) -> p h c", c=97)[:, :, 96:97]
                nc.vector.memset(ones_ap, 1.0)
                for half in range(2):
                    vp = pps.tile([128, 320], FP32, tag="ps", name=f"vp{half}")
                    for kt in range(n_kt):
                        nc.tensor.matmul(
                            vp[0:n_tok, :],
                            r32(n_tiles[kt][:, tok_off:tok_off + n_tok]),
                            r32(w_tiles[kt][:, half * 320:(half + 1) * 320]),
                            start=(kt == 0), stop=(kt == n_kt - 1))
                    dst = vt[:, half * 388:half * 388 + 388].rearrange(
                        "p (h c) -> p h c", c=97)[0:n_tok, :, 0:80]
                    src = vp[0:n_tok, :].rearrange("p (h c) -> p h c", c=80)
                    nc.vector.tensor_copy(dst, src)

            def attention(qT_t, kT_t, v_t, n_keytiles, key_dim_last, aT_t, e_pool):
                """S^T -> exp -> AV; attention output is evicted unnormalized
                and the 16 per-(head, q-half) denominators are batched into
                two 32-row-aligned tiles so just two accurate reciprocals run
                (a [1,512] DVE reciprocal costs ~3.3us; 32 of them dominated
                the v1 profile)."""
                den_t = {}
                denr_t = {}

                def dslot(p):
                    return p // 3, 32 * (p % 3)

                def emit_group_normalize(t):
                    """reciprocal of den tile t + normalize its pairs."""
                    dr = prcb.tile([128, QH], F32R, tag="denr", bufs=3,
                                   name=f"denr{t}")
                    with nc.allow_low_precision(reason="fp32r denom rounding"):
                        nc.vector.reciprocal(dr[:], den_t[t][:])
                    denr_t[t] = dr
                    for p in range(3 * t, min(3 * t + 3, n_pairs)):
                        h, hh = p // 2, p % 2
                        _, drow = dslot(p)
                        rcb = pps.tile([128, QH], FP32, tag="avps", bufs=2,
                                       name=f"rcb{h}{hh}")
                        nc.tensor.matmul(
                            rcb[0:DH, :], onesr[drow:drow + 1, 0:DH],
                            dr[drow:drow + 1, :], start=True, stop=True)
                        seg = aT_t[h][0:DH, hh * QH:(hh + 1) * QH]
                        nc.vector.tensor_tensor(seg, seg, rcb[0:DH, :], OP.mult)
                npairs = (n_keytiles + 1) // 2
                n_pairs = 2 * H
                for h in range(H):
                    at = aT_t[h]
                    # rows 80:128 are padding consumed by the padded out-proj;
                    # zero from 64 (SBUF APs must start at partition 0/32/64/96)
                    nc.vector.memset(at[64:128, :], 0.0)
                    for hh in range(2):
                        p = h * 2 + hh
                        avp = pps.tile([128, QH], FP32, tag="avps", bufs=2,
                                       name=f"av{h}{hh}")
                        # two score tiles share one 2-bank PSUM tile so a
                        # single exp covers both (halves the ACT op count);
                        # pipelined one pair ahead of the AV consumers
                        ets = {}
                        for pt in range(npairs + 1):
                            if pt < npairs:
                                kts = [kt for kt in (2 * pt, 2 * pt + 1)
                                       if kt < n_keytiles]
                                spp = pps.tile([128, 2 * QH], FP32, tag="sps",
                                               bufs=2, name=f"s{h}{hh}{pt}")
                                klens = []
                                for j, kt in enumerate(kts):
                                    klen = (key_dim_last
                                            if kt == n_keytiles - 1 else 128)
                                    klens.append(klen)
                                    nc.tensor.matmul(
                                        spp[0:klen, j * QH:(j + 1) * QH],
                                        kT_t[h][0:DH, kt * 128:kt * 128 + klen],
                                        qT_t[h][0:DH, hh * QH:(hh + 1) * QH],
                                        start=True, stop=True)
                                et = e_pool.tile([128, 2 * QH], BF16, tag="E",
                                                 name=f"e{h}{hh}{pt}")
                                if len(kts) == 2 and klens[0] == klens[1]:
                                    nc.scalar.activation(
                                        et[0:klens[0], :], spp[0:klens[0], :],
                                        AF.Exp, scale=scale)
                                else:
                                    for j, kt in enumerate(kts):
                                        nc.scalar.activation(
                                            et[0:klens[j], j * QH:(j + 1) * QH],
                                            spp[0:klens[j], j * QH:(j + 1) * QH],
                                            AF.Exp, scale=scale)
                                ets[pt] = (et, kts, klens)
                            if pt > 0:
                                pet, pkts, pklens = ets.pop(pt - 1)
                                for j, kt in enumerate(pkts):
                                    nc.tensor.matmul(
                                        avp[0:97, :],
                                        v_t[kt][0:pklens[j], h * 97:(h + 1) * 97],
                                        pet[0:pklens[j], j * QH:(j + 1) * QH],
                                        start=(kt == 0), stop=(kt == n_keytiles - 1))
                        # unnormalized evict (frees the PSUM bank) + denom stash
                        nc.vector.tensor_copy(at[0:DH, hh * QH:(hh + 1) * QH],
                                              avp[0:DH, :])
                        dt_i, drow = dslot(p)
                        if dt_i not in den_t:
                            dn = prcb.tile([128, QH], FP32, tag="den", bufs=3,
                                           name=f"den{dt_i}")
                            nc.vector.memset(dn[:], 1.0)
                            den_t[dt_i] = dn
                        nc.vector.tensor_copy(
                            den_t[dt_i][drow:drow + 1, :], avp[96:97, :])
                        if p == 3 * dt_i + 2 or p == n_pairs - 1:
                            emit_group_normalize(dt_i)


            def out_proj(wp_tiles, aT_t, res_t, bias_off):
                """res += aT @ o^T + bias (in-place residual update)."""
                for m in range(DKT):
                    for hh in range(2):
                        op_ = pps.tile([128, QH], FP32, tag="ps", name=f"op{m}{hh}")
                        for kt in range(H):
                            nc.tensor.matmul(
                                op_[:, :],
                                wp_tiles[kt][:, m * 128:(m + 1) * 128],
                                aT_t[kt][:, hh * QH:(hh + 1) * QH],
                                start=(kt == 0), stop=(kt == H - 1))
                        seg = res_t[m][:, hh * QH:(hh + 1) * QH]
                        nc.vector.scalar_tensor_tensor(
                            seg, op_[:, :], bcol(bias_off + m), seg, OP.add, OP.add)

            # residual stream (feature-major, f32)
            res_tiles = []
            for kt in range(DKT):
                rt = pres.tile([128, T], F32R, tag="res", name=f"res_{kt}")
                nc.sync.dma_start(rt[:], dram["hsT_q"][kt * 128:(kt + 1) * 128, :])
                res_tiles.append(rt)

            with ExitStack() as ctx_abcd:
                pqT = ctx_abcd.enter_context(tc.tile_pool(name="qT", bufs=8))
                paT = ctx_abcd.enter_context(tc.tile_pool(name="aT", bufs=8))

                # ---------- phase A: LN1 + QKV projections ----------
                with ExitStack() as ctx_b:
                    pkT = ctx_b.enter_context(tc.tile_pool(name="kT", bufs=8))
                    pV = ctx_b.enter_context(tc.tile_pool(name="V", bufs=16))
                    pE = ctx_b.enter_context(tc.tile_pool(name="E", bufs=4))

                    kT_tiles = [pkT.tile([128, 2 * T], BF16, tag="kT", name=f"kT_{h}")
                                for h in range(H)]
                    v_tiles = [pV.tile([128, 776], BF16, tag="V", name=f"v_{i}")
                               for i in range(2 * TT)]

                    n_q = emit_ln(res_tiles, 1, [])
                    fr0_tiles = []
                    for kt in range(DKT):
                        ft = pn.tile([128, T], F32R, tag="fr", bufs=5,
                                     name=f"fr0_{kt}")
                        nc.sync.dma_start(
                            ft[:], dram["hsT_first"][kt * 128:(kt + 1) * 128, :])
                        fr0_tiles.append(ft)
                    emit_ln(fr0_tiles, 1, fr0_tiles)  # in place, overlaps Q proj
                    q1_sb = load_w("q1", DKT, "w", pw)
                    qT_tiles = [pqT.tile([128, T], BF16, tag="qT", name=f"qT_{h}")
                                for h in range(H)]
                    head_proj(q1_sb, n_q, qT_tiles, 0, DKT, "q")

                    for fi, fr_tiles in enumerate((fr0_tiles, None)):
                        if fr_tiles is None:
                            fr_tiles = []
                            for kt in range(DKT):
                                ft = pn.tile([128, T], F32R, tag="fr", bufs=5,
                                             name=f"fr1_{kt}")
                                nc.sync.dma_start(
                                    ft[:],
                                    dram["hsT_former"][kt * 128:(kt + 1) * 128, :])
                                fr_tiles.append(ft)
                            emit_ln(fr_tiles, 1, fr_tiles)  # in place
                        k1_sb = load_w("k1", DKT, "w", pw)
                        head_proj(k1_sb, fr_tiles, kT_tiles, fi * T, DKT, f"k{fi}")
                        v1_sb = load_w("v1", DKT, "w", pw)
                        for tt in range(TT):
                            v_proj(fr_tiles, v_tiles[fi * TT + tt], DKT, v1_sb,
                                   128, tt * 128)

                    # ---------- phase B: sparse-causal attention ----------
                    aT_tiles = [paT.tile([128, T], BF16, tag="aT", name=f"aT_{h}")
                                for h in range(H)]
                    attention(qT_tiles, kT_tiles, v_tiles, 2 * TT, 128, aT_tiles, pE)

                # ---------- phase C: o1 + residual ----------
                with ExitStack() as ctx_c:
                    pwp = ctx_c.enter_context(tc.tile_pool(name="wp", bufs=8))
                    o1p_sb = load_w("o1p", H, "wp", pwp, dtype=BF16)
                    out_proj(o1p_sb, aT_tiles, res_tiles, OB1)

                # ---------- phase D: cross attention ----------
                with ExitStack() as ctx_d:
                    penc = ctx_d.enter_context(tc.tile_pool(name="enc", bufs=6))
                    pk2 = ctx_d.enter_context(tc.tile_pool(name="k2T", bufs=8))
                    pV2 = ctx_d.enter_context(tc.tile_pool(name="V2", bufs=1))
                    pE2 = ctx_d.enter_context(tc.tile_pool(name="E2", bufs=4))
                    pwp2 = ctx_d.enter_context(tc.tile_pool(name="wp2", bufs=8))

                    n2 = emit_ln(res_tiles, 2, [])
                    q2_sb = load_w("q2", DKT, "w", pw)
                    q2T_tiles = [pqT.tile([128, T], BF16, tag="qT", name=f"q2T_{h}")
                                 for h in range(H)]
                    head_proj(q2_sb, n2, q2T_tiles, 0, DKT, "q2")

                    enc_tiles = []
                    for kt in range(CKT):
                        et_ = penc.tile([128, CTXP], F32R, tag="enc", name=f"enc_{kt}")
                        nc.sync.dma_start(
                            et_[:], dram["encT"][kt * 128:(kt + 1) * 128, :])
                        enc_tiles.append(et_)
                    k2_sb = load_w("k2", CKT, "w", pw)
                    k2T_tiles = [pk2.tile([128, CTXP], BF16, tag="k2T", name=f"k2T_{h}")
                                 for h in range(H)]
                    for h in range(H):
                        kp = pps.tile([128, CTXP], FP32, tag="ps", name=f"k2p{h}")
                        for kt in range(CKT):
                            nc.tensor.matmul(kp[0:DH, :],
                                             r32(k2_sb[kt][:, h * DH:(h + 1) * DH]),
                                             r32(enc_tiles[kt][:]),
                                             start=(kt == 0), stop=(kt == CKT - 1))
                        nc.vector.tensor_copy(k2T_tiles[h][0:DH, :], kp[0:DH, :])
                    v2_sb = load_w("v2", CKT, "w", pw)
                    v2_t = pV2.tile([128, 776], BF16, tag="V2", name="v2t")
                    v_proj(enc_tiles, v2_t, CKT, v2_sb, CTX, 0)

                    a2T_tiles = [paT.tile([128, T], BF16, tag="aT", name=f"a2T_{h}")
                                 for h in range(H)]
                    attention(q2T_tiles, k2T_tiles, [v2_t], 1, CTX, a2T_tiles, pE2)
                    o2p_sb = load_w("o2p", H, "wp2", pwp2, dtype=BF16)
                    out_proj(o2p_sb, a2T_tiles, res_tiles, OB2)

            # ---------- phase E: GEGLU feed-forward ----------
            with ExitStack() as ctx_e:
                pG = ctx_e.enter_context(tc.tile_pool(name="gT", bufs=20))
                pgl = ctx_e.enter_context(tc.tile_pool(name="gl", bufs=3))
                pff2 = ctx_e.enter_context(tc.tile_pool(name="ff2w", bufs=20))

                n3 = emit_ln(res_tiles, 3, [])
                gT_tiles = []
                for mi in range(FMT):
                    fx = pw.tile([128, D], F32R, tag="w", name=f"fx{mi}")
                    fg = pw.tile([128, D], F32R, tag="w", name=f"fg{mi}")
                    fx_dst = fx[:].rearrange("p (k c) -> p k c", c=128)
                    fg_dst = fg[:].rearrange("p (k c) -> p k c", c=128)
                    fx_src = dram["ff1b"][mi].rearrange("(k p) c -> p k c", p=128)
                    fg_src = dram["ff1b"][FMT + mi].rearrange("(k p) c -> p k c", p=128)
                    nc.sync.dma_start(fx_dst, fx_src)
                    nc.sync.dma_start(fg_dst, fg_src)
                    gt = pG.tile([128, T], BF16, tag="gT", name=f"gT_{mi}")
                    gT_tiles.append(gt)
                    for hh in range(2):
                        xp = pps.tile([128, QH], FP32, tag="ps", name=f"xp{mi}{hh}")
                        gp = pps.tile([128, QH], FP32, tag="ps", name=f"gp{mi}{hh}")
                        for kt in range(DKT):
                            nc.tensor.matmul(
                                xp[:, :], r32(fx[:, kt * 128:(kt + 1) * 128]),
                                r32(n3[kt][:, hh * QH:(hh + 1) * QH]),
                                start=(kt == 0), stop=(kt == DKT - 1))
                        for kt in range(DKT):
                            nc.tensor.matmul(
                                gp[:, :], r32(fg[:, kt * 128:(kt + 1) * 128]),
                                r32(n3[kt][:, hh * QH:(hh + 1) * QH]),
                                start=(kt == 0), stop=(kt == DKT - 1))
                        gl = pgl.tile([128, QH], BF16, tag="gl", name=f"gl{mi}{hh}")
                        nc.scalar.activation(gl[:], gp[:, :], GELU_AF or AF.Gelu,
                                             bias=bcol(FBG + mi), scale=1.0)
                        nc.vector.scalar_tensor_tensor(
                            gt[:, hh * QH:(hh + 1) * QH], xp[:, :], bcol(FBX + mi),
                            gl[:], OP.add, OP.mult)

                ff2_sb = load_w("ff2", FMT, "ff2w", pff2, dtype=BF16)
                for m in range(DKT):
                    for hh in range(2):
                        fp = pps.tile([128, QH], FP32, tag="ps", name=f"fp{m}{hh}")
                        for kt in range(FMT):
                            nc.tensor.matmul(
                                fp[:, :], ff2_sb[kt][:, m * 128:(m + 1) * 128],
                                gT_tiles[kt][:, hh * QH:(hh + 1) * QH],
                                start=(kt == 0), stop=(kt == FMT - 1))
                        seg = res_tiles[m][:, hh * QH:(hh + 1) * QH]
                        nc.vector.scalar_tensor_tensor(
                            seg, fp[:, :], bcol(FB2 + m), seg, OP.add, OP.add)
            for m in range(DKT):
                nc.sync.dma_start(out_dram[m * 128:(m + 1) * 128, :], res_tiles[m][:])

    nc.compile()
    return nc


_PROGRAM_CACHE = {}


def _get_program(ln_trivial):
    key = (tuple(ln_trivial), GELU_AF)
    if key not in _PROGRAM_CACHE:
        _PROGRAM_CACHE[key] = build_program(ln_trivial)
    return _PROGRAM_CACHE[key]


def _pad_heads(w):
    """[640, 640] head rows -> [1024, 640] padded to 128/head."""
    out = np.zeros((H * 128, D), np.float32)
    for h in range(H):
        out[h * 128:h * 128 + DH] = w[h * DH:(h + 1) * DH]
    return out


def _bias_cols(vec, n):
    return np.ascontiguousarray(vec.reshape(n, 128).T)


def kernel(**inputs):
    hs = np.ascontiguousarray(inputs["hidden_states"], np.float32)
    enc = np.ascontiguousarray(inputs["encoder_hidden_states"], np.float32)
    f = int(inputs["video_length"])
    BF = hs.shape[0]
    assert BF == N_CORES and hs.shape[1:] == (T, D)

    ln_trivial = tuple(
        bool(np.all(inputs[f"n{i}_g"] == 1.0) and np.all(inputs[f"n{i}_b"] == 0.0))
        for i in (1, 2, 3))
    nc = _get_program(ln_trivial)

    biases = np.zeros((128, NB), np.float32)
    biases[:, EPS_COL] = LN_EPS
    biases[:, OB1:OB1 + 5] = _bias_cols(inputs["o1_b"].astype(np.float32), 5)
    biases[:, OB2:OB2 + 5] = _bias_cols(inputs["o2_b"].astype(np.float32), 5)
    biases[:, FB2:FB2 + 5] = _bias_cols(inputs["ff2_b"].astype(np.float32), 5)
    ff1_b = inputs["ff1_b"].astype(np.float32)
    biases[:, FBX:FBX + FMT] = _bias_cols(ff1_b[:DFF], FMT)
    biases[:, FBG:FBG + FMT] = _bias_cols(ff1_b[DFF:], FMT)
    for i in (1, 2, 3):
        biases[:, LN_G[i]:LN_G[i] + 5] = _bias_cols(inputs[f"n{i}_g"].astype(np.float32), 5)
        biases[:, LN_B[i]:LN_B[i] + 5] = _bias_cols(inputs[f"n{i}_b"].astype(np.float32), 5)

    ff1 = inputs["ff1"].astype(np.float32)  # [640, 5120]
    ff1b = np.ascontiguousarray(
        ff1.reshape(DKT, 128, 2 * FMT, 128).transpose(2, 0, 1, 3).reshape(2 * FMT, D, 128))

    common = {
        "q1": np.ascontiguousarray(inputs["q1"], np.float32),
        "k1": np.ascontiguousarray(inputs["k1"], np.float32),
        "v1": np.ascontiguousarray(inputs["v1"], np.float32),
        "q2": np.ascontiguousarray(inputs["q2"], np.float32),
        "k2": np.ascontiguousarray(inputs["k2"], np.float32),
        "v2": np.ascontiguousarray(inputs["v2"], np.float32),
        "o1p": _pad_heads(inputs["o1"].astype(np.float32)).astype(ml_dtypes.bfloat16),
        "o2p": _pad_heads(inputs["o2"].astype(np.float32)).astype(ml_dtypes.bfloat16),
        "ff1b": ff1b,
        "ff2": np.ascontiguousarray(inputs["ff2"], np.float32).astype(ml_dtypes.bfloat16),
        "biases": biases,
    }

    hsT = np.ascontiguousarray(hs.transpose(0, 2, 1))      # [BF, 640, 1024]
    encT = np.zeros((BF, CROSS, CTXP), np.float32)         # ctx padded 77 -> 80
    encT[:, :, :CTX] = enc.transpose(0, 2, 1)
    in_maps = []
    for g in range(BF):
        bi, fi = divmod(g, f)
        first = bi * f
        former = bi * f + max(fi - 1, 0)
        in_maps.append({
            **common,
            "hsT_q": hsT[g],
            "hsT_first": hsT[first],
            "hsT_former": hsT[former],
            "encT": encT[g],
        })

    res = run_bass_kernel_spmd(nc, in_maps, core_ids=list(range(N_CORES)),
                               trace=bool(int(os.environ.get("KERNEL_TRACE", "0"))))
    kernel.last_results = res
    out = np.stack([res.results[g]["outT"].T for g in range(BF)])
    return np.ascontiguousarray(out.astype(inputs["hidden_states"].dtype))


# revision 30
# speedup vs baseline: 1.0302x; 1.0302x over previous
"""Trainium2 Bass kernel for a video-diffusion BasicTransformerBlock
(sparse-causal self-attn + cross-attn + GEGLU FF).

Sharding: data-parallel, one (batch, frame) pair per NeuronCore (8 frames ->
8 cores). Each core receives its own frame, frame 0 of its batch, and the
previous frame (duplicated inputs), so the sparse-causal KV gather needs no
collectives. For frames 0/1 the first/former KV frames coincide; softmax over
duplicated keys is mathematically identical to the reference's concat.

On-device layout: activations are feature-major (x^T, [dim, tokens]) so every
projection contracts over SBUF partitions without any transposes. LayerNorm
column-stats come from ones-matmuls; softmax runs max-free (scores are
bounded ~|5.5|) with denominators from an appended ones-column in V.
All transposes happen host-side in numpy.
"""
import os
import sys
import numpy as np

if not os.environ.get("TRN_TERMINAL_POOL_IPS"):
    raise RuntimeError("expected axon trn environment")
for _p in ("/opt/trn_rl_repo",):
    if _p not in sys.path:
        sys.path.append(_p)

import ml_dtypes
import concourse.bass as bass
import concourse.tile as tile
from concourse import bacc, mybir
from concourse.bass_utils import run_bass_kernel_spmd

FP32 = mybir.dt.float32
F32R = mybir.dt.float32r
BF16 = mybir.dt.bfloat16
AF = mybir.ActivationFunctionType
OP = mybir.AluOpType

D = 640          # model dim
T = 1024         # tokens / frame
H = 8            # heads
DH = 80          # head dim
DKT = D // 128   # 5 feature tiles of the model dim
TT = T // 128    # 8 token tiles / frame
QH = 512         # query half width
CROSS = 768
CKT = CROSS // 128
CTX = 77
CTXP = 80   # context padded for fp32r free-dim alignment
DFF = 2560       # ff hidden (per GEGLU half)
FMT = DFF // 128  # 20 ff row tiles per half
LN_EPS = 1e-5

# bias-pack column offsets ([128, NB] f32)
OB1, OB2, FB2, FBX, FBG = 0, 5, 10, 15, 35
LN_G = {1: 55, 2: 65, 3: 75}
LN_B = {1: 60, 2: 70, 3: 80}
EPS_COL = 85
NB = 86

N_CORES = 8

# test hook: CoreSim lacks Gelu; tests may override with a sim-supported func
GELU_AF = None


def r32(ap):
    return ap if ap.dtype == F32R else ap.bitcast(F32R)


def build_program(ln_trivial):
    nc = bacc.Bacc("TRN2", target_bir_lowering=False, debug=False,
                   num_devices=N_CORES)
    dram = {}
    for name in ("hsT_q", "hsT_first", "hsT_former"):
        dram[name] = nc.dram_tensor(name, [D, T], F32R, kind="ExternalInput").ap()
    dram["encT"] = nc.dram_tensor("encT", [CROSS, CTXP], F32R, kind="ExternalInput").ap()
    for name in ("q1", "k1", "v1", "q2"):
        dram[name] = nc.dram_tensor(name, [D, D], F32R, kind="ExternalInput").ap()
    for name in ("k2", "v2"):
        dram[name] = nc.dram_tensor(name, [CROSS, D], F32R, kind="ExternalInput").ap()
    for name in ("o1p", "o2p"):
        dram[name] = nc.dram_tensor(name, [H * 128, D], BF16, kind="ExternalInput").ap()
    dram["ff1b"] = nc.dram_tensor("ff1b", [2 * FMT, D, 128], F32R, kind="ExternalInput").ap()
    dram["ff2"] = nc.dram_tensor("ff2", [DFF, D], BF16, kind="ExternalInput").ap()
    dram["biases"] = nc.dram_tensor("biases", [128, NB], FP32, kind="ExternalInput").ap()
    out_dram = nc.dram_tensor("outT", [D, T], F32R, kind="ExternalOutput").ap()

    scale = float(DH) ** -0.5

    with tile.TileContext(nc) as tc:
        from contextlib import ExitStack
        with ExitStack() as ctx:
            pc = ctx.enter_context(tc.tile_pool(name="const", bufs=1))
            pres = ctx.enter_context(tc.tile_pool(name="res", bufs=5))
            pn = ctx.enter_context(tc.tile_pool(name="n", bufs=5))
            psq = ctx.enter_context(tc.tile_pool(name="sq", bufs=1))
            prow = ctx.enter_context(tc.tile_pool(name="row", bufs=1))
            pbc = ctx.enter_context(tc.tile_pool(name="bc", bufs=2))
            prcb = ctx.enter_context(tc.tile_pool(name="rcb", bufs=2))
            pw = ctx.enter_context(tc.tile_pool(name="w", bufs=6))
            pps = ctx.enter_context(tc.tile_pool(name="ps", bufs=2, space="PSUM"))

            bias_sb = pc.tile([128, NB], FP32, tag="bias")
            nc.sync.dma_start(bias_sb[:], dram["biases"][:])
            invd_f = pc.tile([128, 1], FP32, tag="invdf")
            nc.vector.memset(invd_f[:], 1.0 / D)
            invd = pc.tile([128, 1], F32R, tag="invd")
            nc.vector.tensor_copy(invd[:], invd_f[:])  # fp32r rounding producer
            onesr_f = pc.tile([128, 128], FP32, tag="onesrf")
            nc.vector.memset(onesr_f[:], 1.0)
            onesr = pc.tile([128, 128], F32R, tag="onesr")
            nc.vector.tensor_copy(onesr[:], onesr_f[:])

            def bcol(j):
                return bias_sb[:, j:j + 1]

            def load_w(dname, n_kt, tag, pool, dtype=F32R):
                tiles = []
                for kt in range(n_kt):
                    wt = pool.tile([128, D], dtype, tag=tag, name=f"{dname}_{kt}")
                    nc.sync.dma_start(wt[:], dram[dname][kt * 128:(kt + 1) * 128, :])
                    tiles.append(wt)
                return tiles

            def emit_ln(x_tiles, which, out_tiles):
                """Feature-major LN of 5 [128, T] fp32r tiles.

                Column stats via fp32r ones-matmuls; mean/rstd rows for the
                two query halves are packed at partitions 0/32 so one batched
                DVE reciprocal serves both, and broadcasting across
                partitions is a PE ones-column outer product into PSUM
                (gpsimd partition_broadcast corrupts offset-row sources on
                HW). out_tiles: list that receives the 5 result APs; passing
                x_tiles itself runs the LN in place."""
                in_place = out_tiles is x_tiles
                mup = prow.tile([128, QH], F32R, tag="mup", bufs=2, name=f"mup{which}")
                msqp = prow.tile([128, QH], FP32, tag="msqp", bufs=2, name=f"msqp{which}")
                rstd = prow.tile([128, QH], F32R, tag="rstd", bufs=2, name=f"rstd{which}")
                mu_b = {}
                for hh in range(2):
                    sl = slice(hh * QH, (hh + 1) * QH)
                    r0 = 32 * hh
                    stp = pps.tile([128, 2 * QH], FP32, tag="sps", bufs=2,
                                   name=f"lnps{which}{hh}")
                    sp = stp[:, 0:QH]
                    spq = stp[:, QH:2 * QH]
                    for kt in range(DKT):
                        nc.tensor.matmul(sp[0:1, :], invd[:, 0:1],
                                         x_tiles[kt][:, sl],
                                         start=(kt == 0), stop=(kt == DKT - 1))
                    for kt in range(DKT):
                        sq = psq.tile([128, QH], F32R, tag="sq", name=f"sq{which}{hh}{kt}")
                        nc.scalar.square(sq[:], x_tiles[kt][:, sl])
                        nc.tensor.matmul(spq[0:1, :], invd[:, 0:1], sq[:],
                                         start=(kt == 0), stop=(kt == DKT - 1))
                    nc.vector.tensor_copy(mup[r0:r0 + 1, :], sp[0:1, :])
                    nc.vector.tensor_copy(msqp[r0:r0 + 1, :], spq[0:1, :])
                    mb = pps.tile([128, QH], FP32, tag="avps", bufs=2,
                                  name=f"mub{which}{hh}")
                    nc.tensor.matmul(mb[:, :], onesr[r0:r0 + 1, :],
                                     mup[r0:r0 + 1, :], start=True, stop=True)
                    mu_b[hh] = mb
                    # pass 1: x - mu (frees the mu broadcast PSUM bank early)
                    for kt in range(DKT):
                        if in_place:
                            nt_seg = x_tiles[kt][:, sl]
                        else:
                            if hh == 0:
                                nt = pn.tile([128, T], F32R, tag="n",
                                             name=f"n{which}_{kt}")
                                out_tiles.append(nt)
                            nt_seg = out_tiles[kt][:, sl]
                        nc.vector.tensor_tensor(nt_seg, x_tiles[kt][:, sl],
                                                mu_b[hh][:, :], OP.subtract)
                    # -var = mu^2 - E[x^2] at the packed row
                    nc.vector.tensor_tensor(mup[r0:r0 + 1, :], mup[r0:r0 + 1, :],
                                            mup[r0:r0 + 1, :], OP.mult)
                    nc.vector.tensor_tensor(mup[r0:r0 + 1, :], mup[r0:r0 + 1, :],
                                            msqp[r0:r0 + 1, :], OP.subtract)
                    # rstd = exp(-0.5 * ln(var + eps)); ACT Ln/Exp round trip
                    # measured at 1.1e-5 max rel on HW, and keeps the whole
                    # tail off the (busier) vector engine
                    nc.scalar.activation(msqp[r0:r0 + 1, :], mup[r0:r0 + 1, :],
                                         AF.Ln, scale=-1.0,
                                         bias=bias_sb[0:1, EPS_COL:EPS_COL + 1])
                    nc.scalar.activation(rstd[r0:r0 + 1, :], msqp[r0:r0 + 1, :],
                                         AF.Exp, scale=-0.5)
                for hh in range(2):
                    sl = slice(hh * QH, (hh + 1) * QH)
                    r0 = 32 * hh
                    rb = pps.tile([128, QH], FP32, tag="avps", bufs=2,
                                  name=f"rb{which}{hh}")
                    nc.tensor.matmul(rb[:, :], onesr[r0:r0 + 1, :],
                                     rstd[r0:r0 + 1, :], start=True, stop=True)
                    for kt in range(DKT):
                        nt_seg = (x_tiles[kt] if in_place else out_tiles[kt])[:, sl]
                        nc.vector.tensor_tensor(nt_seg, nt_seg, rb[:, :], OP.mult)
                        if not ln_trivial[which - 1]:
                            nc.scalar.activation(nt_seg, nt_seg, AF.Identity,
                                                 bias=bcol(LN_B[which] + kt),
                                                 scale=bcol(LN_G[which] + kt))
                return out_tiles

            def head_proj(w_tiles, n_tiles, out_tiles, col_off, n_kt, tag):
                """out^T[h][0:80, col_off:col_off+T] = w.T @ n, per-head padded."""
                for h in range(H):
                    for hh in range(2):
                        qp = pps.tile([128, QH], FP32, tag="ps", name=f"hp{tag}{h}{hh}")
                        for kt in range(n_kt):
                            nc.tensor.matmul(
                                qp[0:DH, :],
                                r32(w_tiles[kt][:, h * DH:(h + 1) * DH]),
                                r32(n_tiles[kt][:, hh * QH:(hh + 1) * QH]),
                                start=(kt == 0), stop=(kt == n_kt - 1))
                        nc.vector.tensor_copy(
                            out_tiles[h][0:DH, col_off + hh * QH:col_off + (hh + 1) * QH],
                            qp[0:DH, :])

            def v_proj(n_tiles, vt, n_kt, w_tiles, n_tok, tok_off):
                """token-major V tile, per-head 97-col slots: data cols 0:80,
                ones col at 96 so the AV denominator lands on PSUM partition
                96 (engine APs must start at partition 0/32/64/96)."""
                pad_ap = vt[:, 0:776].rearrange("p (h c) -> p h c", c=97)[:, :, 80:96]
                nc.vector.memset(pad_ap, 0.0)
                ones_ap = vt[:, 0:776].rearrange("p (h c) -> p h c", c=97)[:, :, 96:97]
                nc.vector.memset(ones_ap, 1.0)
                vpp = pps.tile([128, 2 * QH], FP32, tag="sps", bufs=2, name="vpp")
                for half in range(2):
                    vp = vpp[:, half * QH:half * QH + 320]
                    for kt in range(n_kt):
                        nc.tensor.matmul(
                            vp[0:n_tok, :],
                            r32(n_tiles[kt][:, tok_off:tok_off + n_tok]),
                            r32(w_tiles[kt][:, half * 320:(half + 1) * 320]),
                            start=(kt == 0), stop=(kt == n_kt - 1))
                    dst = vt[:, half * 388:half * 388 + 388].rearrange(
                        "p (h c) -> p h c", c=97)[0:n_tok, :, 0:80]
                    src = vp[0:n_tok, :].rearrange("p (h c) -> p h c", c=80)
                    nc.vector.tensor_copy(dst, src)

            def attention(qT_t, kT_t, v_t, n_keytiles, key_dim_last, aT_t, e_pool):
                """S^T -> exp -> AV; attention output is evicted unnormalized
                and the 16 per-(head, q-half) denominators are batched into
                two 32-row-aligned tiles so just two accurate reciprocals run
                (a [1,512] DVE reciprocal costs ~3.3us; 32 of them dominated
                the v1 profile)."""
                den_t = {}
                denr_t = {}

                def dslot(p):
                    return p // 3, 32 * (p % 3)

                def emit_group_normalize(t):
                    """reciprocal of den tile t + normalize its pairs."""
                    dr = prcb.tile([128, QH], F32R, tag="denr", bufs=3,
                                   name=f"denr{t}")
                    with nc.allow_low_precision(reason="fp32r denom rounding"):
                        nc.vector.reciprocal(dr[:], den_t[t][:])
                    denr_t[t] = dr
                    for p in range(3 * t, min(3 * t + 3, n_pairs)):
                        h, hh = p // 2, p % 2
                        _, drow = dslot(p)
                        rcb = pps.tile([128, QH], FP32, tag="avps", bufs=2,
                                       name=f"rcb{h}{hh}")
                        nc.tensor.matmul(
                            rcb[0:DH, :], onesr[drow:drow + 1, 0:DH],
                            dr[drow:drow + 1, :], start=True, stop=True)
                        seg = aT_t[h][0:DH, hh * QH:(hh + 1) * QH]
                        nc.vector.tensor_tensor(seg, seg, rcb[0:DH, :], OP.mult)
                npairs = (n_keytiles + 1) // 2
                n_pairs = 2 * H
                for h in range(H):
                    at = aT_t[h]
                    # rows 80:128 are padding consumed by the padded out-proj;
                    # zero from 64 (SBUF APs must start at partition 0/32/64/96)
                    nc.vector.memset(at[64:128, :], 0.0)
                    for hh in range(2):
                        p = h * 2 + hh
                        avp = pps.tile([128, QH], FP32, tag="avps", bufs=2,
                                       name=f"av{h}{hh}")
                        # two score tiles share one 2-bank PSUM tile so a
                        # single exp covers both (halves the ACT op count);
                        # pipelined one pair ahead of the AV consumers
                        ets = {}
                        for pt in range(npairs + 1):
                            if pt < npairs:
                                kts = [kt for kt in (2 * pt, 2 * pt + 1)
                                       if kt < n_keytiles]
                                spp = pps.tile([128, 2 * QH], FP32, tag="sps",
                                               bufs=2, name=f"s{h}{hh}{pt}")
                                klens = []
                                for j, kt in enumerate(kts):
                                    klen = (key_dim_last
                                            if kt == n_keytiles - 1 else 128)
                                    klens.append(klen)
                                    nc.tensor.matmul(
                                        spp[0:klen, j * QH:(j + 1) * QH],
                                        kT_t[h][0:DH, kt * 128:kt * 128 + klen],
                                        qT_t[h][0:DH, hh * QH:(hh + 1) * QH],
                                        start=True, stop=True)
                                et = e_pool.tile([128, 2 * QH], BF16, tag="E",
                                                 name=f"e{h}{hh}{pt}")
                                if len(kts) == 2 and klens[0] == klens[1]:
                                    nc.scalar.activation(
                                        et[0:klens[0], :], spp[0:klens[0], :],
                                        AF.Exp, scale=scale)
                                else:
                                    for j, kt in enumerate(kts):
                                        nc.scalar.activation(
                                            et[0:klens[j], j * QH:(j + 1) * QH],
                                            spp[0:klens[j], j * QH:(j + 1) * QH],
                                            AF.Exp, scale=scale)
                                ets[pt] = (et, kts, klens)
                            if pt > 0:
                                pet, pkts, pklens = ets.pop(pt - 1)
                                for j, kt in enumerate(pkts):
                                    nc.tensor.matmul(
                                        avp[0:97, :],
                                        v_t[kt][0:pklens[j], h * 97:(h + 1) * 97],
                                        pet[0:pklens[j], j * QH:(j + 1) * QH],
                                        start=(kt == 0), stop=(kt == n_keytiles - 1))
                        # unnormalized evict (frees the PSUM bank) + denom stash
                        nc.vector.tensor_copy(at[0:DH, hh * QH:(hh + 1) * QH],
                                              avp[0:DH, :])
                        dt_i, drow = dslot(p)
                        if dt_i not in den_t:
                            dn = prcb.tile([128, QH], FP32, tag="den", bufs=3,
                                           name=f"den{dt_i}")
                            nc.vector.memset(dn[:], 1.0)
                            den_t[dt_i] = dn
                        nc.vector.tensor_copy(
                            den_t[dt_i][drow:drow + 1, :], avp[96:97, :])
                        if p == 3 * dt_i + 2 or p == n_pairs - 1:
                            emit_group_normalize(dt_i)


            def out_proj(wp_tiles, aT_t, res_t, bias_off):
                """res += aT @ o^T + bias (in-place residual update)."""
                for m in range(DKT):
                    for hh in range(2):
                        op_ = pps.tile([128, QH], FP32, tag="ps", name=f"op{m}{hh}")
                        for kt in range(H):
                            nc.tensor.matmul(
                                op_[:, :],
                                wp_tiles[kt][:, m * 128:(m + 1) * 128],
                                aT_t[kt][:, hh * QH:(hh + 1) * QH],
                                start=(kt == 0), stop=(kt == H - 1))
                        seg = res_t[m][:, hh * QH:(hh + 1) * QH]
                        nc.vector.scalar_tensor_tensor(
                            seg, op_[:, :], bcol(bias_off + m), seg, OP.add, OP.add)

            # residual stream (feature-major, f32)
            res_tiles = []
            for kt in range(DKT):
                rt = pres.tile([128, T], F32R, tag="res", name=f"res_{kt}")
                nc.sync.dma_start(rt[:], dram["hsT_q"][kt * 128:(kt + 1) * 128, :])
                res_tiles.append(rt)

            with ExitStack() as ctx_abcd:
                pqT = ctx_abcd.enter_context(tc.tile_pool(name="qT", bufs=8))
                paT = ctx_abcd.enter_context(tc.tile_pool(name="aT", bufs=8))

                # ---------- phase A: LN1 + QKV projections ----------
                with ExitStack() as ctx_b:
                    pkT = ctx_b.enter_context(tc.tile_pool(name="kT", bufs=8))
                    pV = ctx_b.enter_context(tc.tile_pool(name="V", bufs=16))
                    pE = ctx_b.enter_context(tc.tile_pool(name="E", bufs=4))

                    kT_tiles = [pkT.tile([128, 2 * T], BF16, tag="kT", name=f"kT_{h}")
                                for h in range(H)]
                    v_tiles = [pV.tile([128, 776], BF16, tag="V", name=f"v_{i}")
                               for i in range(2 * TT)]

                    n_q = emit_ln(res_tiles, 1, [])
                    fr0_tiles = []
                    for kt in range(DKT):
                        ft = pn.tile([128, T], F32R, tag="fr", bufs=5,
                                     name=f"fr0_{kt}")
                        nc.sync.dma_start(
                            ft[:], dram["hsT_first"][kt * 128:(kt + 1) * 128, :])
                        fr0_tiles.append(ft)
                    emit_ln(fr0_tiles, 1, fr0_tiles)  # in place, overlaps Q proj
                    q1_sb = load_w("q1", DKT, "w", pw)
                    qT_tiles = [pqT.tile([128, T], BF16, tag="qT", name=f"qT_{h}")
                                for h in range(H)]
                    head_proj(q1_sb, n_q, qT_tiles, 0, DKT, "q")

                    for fi, fr_tiles in enumerate((fr0_tiles, None)):
                        if fr_tiles is None:
                            fr_tiles = []
                            for kt in range(DKT):
                                ft = pn.tile([128, T], F32R, tag="fr", bufs=5,
                                             name=f"fr1_{kt}")
                                nc.sync.dma_start(
                                    ft[:],
                                    dram["hsT_former"][kt * 128:(kt + 1) * 128, :])
                                fr_tiles.append(ft)
                            emit_ln(fr_tiles, 1, fr_tiles)  # in place
                        k1_sb = load_w("k1", DKT, "w", pw)
                        head_proj(k1_sb, fr_tiles, kT_tiles, fi * T, DKT, f"k{fi}")
                        v1_sb = load_w("v1", DKT, "w", pw)
                        for tt in range(TT):
                            v_proj(fr_tiles, v_tiles[fi * TT + tt], DKT, v1_sb,
                                   128, tt * 128)

                    # ---------- phase B: sparse-causal attention ----------
                    aT_tiles = [paT.tile([128, T], BF16, tag="aT", name=f"aT_{h}")
                                for h in range(H)]
                    attention(qT_tiles, kT_tiles, v_tiles, 2 * TT, 128, aT_tiles, pE)

                # ---------- phase C: o1 + residual ----------
                with ExitStack() as ctx_c:
                    pwp = ctx_c.enter_context(tc.tile_pool(name="wp", bufs=8))
                    o1p_sb = load_w("o1p", H, "wp", pwp, dtype=BF16)
                    out_proj(o1p_sb, aT_tiles, res_tiles, OB1)

                # ---------- phase D: cross attention ----------
                with ExitStack() as ctx_d:
                    penc = ctx_d.enter_context(tc.tile_pool(name="enc", bufs=6))
                    pk2 = ctx_d.enter_context(tc.tile_pool(name="k2T", bufs=8))
                    pV2 = ctx_d.enter_context(tc.tile_pool(name="V2", bufs=1))
                    pE2 = ctx_d.enter_context(tc.tile_pool(name="E2", bufs=4))
                    pwp2 = ctx_d.enter_context(tc.tile_pool(name="wp2", bufs=8))

                    n2 = emit_ln(res_tiles, 2, [])
                    q2_sb = load_w("q2", DKT, "w", pw)
                    q2T_tiles = [pqT.tile([128, T], BF16, tag="qT", name=f"q2T_{h}")
                                 for h in range(H)]
                    head_proj(q2_sb, n2, q2T_tiles, 0, DKT, "q2")

                    enc_tiles = []
                    for kt in range(CKT):
                        et_ = penc.tile([128, CTXP], F32R, tag="enc", name=f"enc_{kt}")
                        nc.sync.dma_start(
                            et_[:], dram["encT"][kt * 128:(kt + 1) * 128, :])
                        enc_tiles.append(et_)
                    k2_sb = load_w("k2", CKT, "w", pw)
                    k2T_tiles = [pk2.tile([128, CTXP], BF16, tag="k2T", name=f"k2T_{h}")
                                 for h in range(H)]
                    for h in range(H):
                        kp = pps.tile([128, CTXP], FP32, tag="ps", name=f"k2p{h}")
                        for kt in range(CKT):
                            nc.tensor.matmul(kp[0:DH, :],
                                             r32(k2_sb[kt][:, h * DH:(h + 1) * DH]),
                                             r32(enc_tiles[kt][:]),
                                             start=(kt == 0), stop=(kt == CKT - 1))
                        nc.vector.tensor_copy(k2T_tiles[h][0:DH, :], kp[0:DH, :])
                    v2_sb = load_w("v2", CKT, "w", pw)
                    v2_t = pV2.tile([128, 776], BF16, tag="V2", name="v2t")
                    v_proj(enc_tiles, v2_t, CKT, v2_sb, CTX, 0)

                    a2T_tiles = [paT.tile([128, T], BF16, tag="aT", name=f"a2T_{h}")
                                 for h in range(H)]
                    attention(q2T_tiles, k2T_tiles, [v2_t], 1, CTX, a2T_tiles, pE2)
                    o2p_sb = load_w("o2p", H, "wp2", pwp2, dtype=BF16)
                    out_proj(o2p_sb, a2T_tiles, res_tiles, OB2)

            # ---------- phase E: GEGLU feed-forward ----------
            with ExitStack() as ctx_e:
                pG = ctx_e.enter_context(tc.tile_pool(name="gT", bufs=20))
                pgl = ctx_e.enter_context(tc.tile_pool(name="gl", bufs=3))
                pff2 = ctx_e.enter_context(tc.tile_pool(name="ff2w", bufs=20))

                n3 = emit_ln(res_tiles, 3, [])
                gT_tiles = []
                for mi in range(FMT):
                    fx = pw.tile([128, D], F32R, tag="w", name=f"fx{mi}")
                    fg = pw.tile([128, D], F32R, tag="w", name=f"fg{mi}")
                    fx_dst = fx[:].rearrange("p (k c) -> p k c", c=128)
                    fg_dst = fg[:].rearrange("p (k c) -> p k c", c=128)
                    fx_src = dram["ff1b"][mi].rearrange("(k p) c -> p k c", p=128)
                    fg_src = dram["ff1b"][FMT + mi].rearrange("(k p) c -> p k c", p=128)
                    nc.sync.dma_start(fx_dst, fx_src)
                    nc.sync.dma_start(fg_dst, fg_src)
                    gt = pG.tile([128, T], BF16, tag="gT", name=f"gT_{mi}")
                    gT_tiles.append(gt)
                    for hh in range(2):
                        xgp = pps.tile([128, 2 * QH], FP32, tag="sps", bufs=2,
                                       name=f"xgp{mi}{hh}")
                        xp = xgp[:, 0:QH]
                        gp = xgp[:, QH:2 * QH]
                        for kt in range(DKT):
                            nc.tensor.matmul(
                                xp[:, :], r32(fx[:, kt * 128:(kt + 1) * 128]),
                                r32(n3[kt][:, hh * QH:(hh + 1) * QH]),
                                start=(kt == 0), stop=(kt == DKT - 1))
                        for kt in range(DKT):
                            nc.tensor.matmul(
                                gp[:, :], r32(fg[:, kt * 128:(kt + 1) * 128]),
                                r32(n3[kt][:, hh * QH:(hh + 1) * QH]),
                                start=(kt == 0), stop=(kt == DKT - 1))
                        gl = pgl.tile([128, QH], BF16, tag="gl", name=f"gl{mi}{hh}")
                        nc.scalar.activation(gl[:], gp[:, :], GELU_AF or AF.Gelu,
                                             bias=bcol(FBG + mi), scale=1.0)
                        nc.vector.scalar_tensor_tensor(
                            gt[:, hh * QH:(hh + 1) * QH], xp[:, :], bcol(FBX + mi),
                            gl[:], OP.add, OP.mult)

                ff2_sb = load_w("ff2", FMT, "ff2w", pff2, dtype=BF16)
                for m in range(DKT):
                    for hh in range(2):
                        fp = pps.tile([128, QH], FP32, tag="ps", name=f"fp{m}{hh}")
                        for kt in range(FMT):
                            nc.tensor.matmul(
                                fp[:, :], ff2_sb[kt][:, m * 128:(m + 1) * 128],
                                gT_tiles[kt][:, hh * QH:(hh + 1) * QH],
                                start=(kt == 0), stop=(kt == FMT - 1))
                        seg = res_tiles[m][:, hh * QH:(hh + 1) * QH]
                        nc.vector.scalar_tensor_tensor(
                            seg, fp[:, :], bcol(FB2 + m), seg, OP.add, OP.add)
            for m in range(DKT):
                nc.sync.dma_start(out_dram[m * 128:(m + 1) * 128, :], res_tiles[m][:])

    nc.compile()
    return nc


_PROGRAM_CACHE = {}


def _get_program(ln_trivial):
    key = (tuple(ln_trivial), GELU_AF)
    if key not in _PROGRAM_CACHE:
        _PROGRAM_CACHE[key] = build_program(ln_trivial)
    return _PROGRAM_CACHE[key]


def _pad_heads(w):
    """[640, 640] head rows -> [1024, 640] padded to 128/head."""
    out = np.zeros((H * 128, D), np.float32)
    for h in range(H):
        out[h * 128:h * 128 + DH] = w[h * DH:(h + 1) * DH]
    return out


def _bias_cols(vec, n):
    return np.ascontiguousarray(vec.reshape(n, 128).T)


def kernel(**inputs):
    hs = np.ascontiguousarray(inputs["hidden_states"], np.float32)
    enc = np.ascontiguousarray(inputs["encoder_hidden_states"], np.float32)
    f = int(inputs["video_length"])
    BF = hs.shape[0]
    assert BF == N_CORES and hs.shape[1:] == (T, D)

    ln_trivial = tuple(
        bool(np.all(inputs[f"n{i}_g"] == 1.0) and np.all(inputs[f"n{i}_b"] == 0.0))
        for i in (1, 2, 3))
    nc = _get_program(ln_trivial)

    biases = np.zeros((128, NB), np.float32)
    biases[:, EPS_COL] = LN_EPS
    biases[:, OB1:OB1 + 5] = _bias_cols(inputs["o1_b"].astype(np.float32), 5)
    biases[:, OB2:OB2 + 5] = _bias_cols(inputs["o2_b"].astype(np.float32), 5)
    biases[:, FB2:FB2 + 5] = _bias_cols(inputs["ff2_b"].astype(np.float32), 5)
    ff1_b = inputs["ff1_b"].astype(np.float32)
    biases[:, FBX:FBX + FMT] = _bias_cols(ff1_b[:DFF], FMT)
    biases[:, FBG:FBG + FMT] = _bias_cols(ff1_b[DFF:], FMT)
    for i in (1, 2, 3):
        biases[:, LN_G[i]:LN_G[i] + 5] = _bias_cols(inputs[f"n{i}_g"].astype(np.float32), 5)
        biases[:, LN_B[i]:LN_B[i] + 5] = _bias_cols(inputs[f"n{i}_b"].astype(np.float32), 5)

    ff1 = inputs["ff1"].astype(np.float32)  # [640, 5120]
    ff1b = np.ascontiguousarray(
        ff1.reshape(DKT, 128, 2 * FMT, 128).transpose(2, 0, 1, 3).reshape(2 * FMT, D, 128))

    common = {
        "q1": np.ascontiguousarray(inputs["q1"], np.float32),
        "k1": np.ascontiguousarray(inputs["k1"], np.float32),
        "v1": np.ascontiguousarray(inputs["v1"], np.float32),
        "q2": np.ascontiguousarray(inputs["q2"], np.float32),
        "k2": np.ascontiguousarray(inputs["k2"], np.float32),
        "v2": np.ascontiguousarray(inputs["v2"], np.float32),
        "o1p": _pad_heads(inputs["o1"].astype(np.float32)).astype(ml_dtypes.bfloat16),
        "o2p": _pad_heads(inputs["o2"].astype(np.float32)).astype(ml_dtypes.bfloat16),
        "ff1b": ff1b,
        "ff2": np.ascontiguousarray(inputs["ff2"], np.float32).astype(ml_dtypes.bfloat16),
        "biases": biases,
    }

    hsT = np.ascontiguousarray(hs.transpose(0, 2, 1))      # [BF, 640, 1024]
    encT = np.zeros((BF, CROSS, CTXP), np.float32)         # ctx padded 77 -> 80
    encT[:, :, :CTX] = enc.transpose(0, 2, 1)
    in_maps = []
    for g in range(BF):
        bi, fi = divmod(g, f)
        first = bi * f
        former = bi * f + max(fi - 1, 0)
        in_maps.append({
            **common,
            "hsT_q": hsT[g],
            "hsT_first": hsT[first],
            "hsT_former": hsT[former],
            "encT": encT[g],
        })

    res = run_bass_kernel_spmd(nc, in_maps, core_ids=list(range(N_CORES)),
                               trace=bool(int(os.environ.get("KERNEL_TRACE", "0"))))
    kernel.last_results = res
    out = np.stack([res.results[g]["outT"].T for g in range(BF)])
    return np.ascontiguousarray(out.astype(inputs["hidden_states"].dtype))


# revision 31
# speedup vs baseline: 1.0531x; 1.0222x over previous
"""Trainium2 Bass kernel for a video-diffusion BasicTransformerBlock
(sparse-causal self-attn + cross-attn + GEGLU FF).

Sharding: data-parallel, one (batch, frame) pair per NeuronCore (8 frames ->
8 cores). Each core receives its own frame, frame 0 of its batch, and the
previous frame (duplicated inputs), so the sparse-causal KV gather needs no
collectives. For frames 0/1 the first/former KV frames coincide; softmax over
duplicated keys is mathematically identical to the reference's concat.

On-device layout: activations are feature-major (x^T, [dim, tokens]) so every
projection contracts over SBUF partitions without any transposes. LayerNorm
column-stats come from ones-matmuls; softmax runs max-free (scores are
bounded ~|5.5|) with denominators from an appended ones-column in V.
All transposes happen host-side in numpy.
"""
import os
import sys
import numpy as np

if not os.environ.get("TRN_TERMINAL_POOL_IPS"):
    raise RuntimeError("expected axon trn environment")
for _p in ("/opt/trn_rl_repo",):
    if _p not in sys.path:
        sys.path.append(_p)

import ml_dtypes
import concourse.bass as bass
import concourse.tile as tile
from concourse import bacc, mybir
from concourse.bass_utils import run_bass_kernel_spmd

FP32 = mybir.dt.float32
F32R = mybir.dt.float32r
BF16 = mybir.dt.bfloat16
AF = mybir.ActivationFunctionType
OP = mybir.AluOpType

D = 640          # model dim
T = 1024         # tokens / frame
H = 8            # heads
DH = 80          # head dim
DKT = D // 128   # 5 feature tiles of the model dim
TT = T // 128    # 8 token tiles / frame
QH = 512         # query half width
CROSS = 768
CKT = CROSS // 128
CTX = 77
CTXP = 80   # context padded for fp32r free-dim alignment
DFF = 2560       # ff hidden (per GEGLU half)
FMT = DFF // 128  # 20 ff row tiles per half
LN_EPS = 1e-5

# bias-pack column offsets ([128, NB] f32)
OB1, OB2, FB2, FBX, FBG = 0, 5, 10, 15, 35
LN_G = {1: 55, 2: 65, 3: 75}
LN_B = {1: 60, 2: 70, 3: 80}
EPS_COL = 85
NB = 86

N_CORES = 8

# test hook: CoreSim lacks Gelu; tests may override with a sim-supported func
GELU_AF = None


def r32(ap):
    return ap if ap.dtype == F32R else ap.bitcast(F32R)


def build_program(ln_trivial):
    nc = bacc.Bacc("TRN2", target_bir_lowering=False, debug=False,
                   num_devices=N_CORES)
    dram = {}
    for name in ("hsT_q", "hsT_first", "hsT_former"):
        dram[name] = nc.dram_tensor(name, [D, T], F32R, kind="ExternalInput").ap()
    dram["encT"] = nc.dram_tensor("encT", [CROSS, CTXP], F32R, kind="ExternalInput").ap()
    for name in ("q1", "k1", "v1", "q2"):
        dram[name] = nc.dram_tensor(name, [D, D], F32R, kind="ExternalInput").ap()
    for name in ("k2", "v2"):
        dram[name] = nc.dram_tensor(name, [CROSS, D], F32R, kind="ExternalInput").ap()
    for name in ("o1p", "o2p"):
        dram[name] = nc.dram_tensor(name, [H * 128, D], BF16, kind="ExternalInput").ap()
    dram["ff1b"] = nc.dram_tensor("ff1b", [2 * FMT, D, 128], F32R, kind="ExternalInput").ap()
    dram["ff2"] = nc.dram_tensor("ff2", [DFF, D], BF16, kind="ExternalInput").ap()
    dram["biases"] = nc.dram_tensor("biases", [128, NB], FP32, kind="ExternalInput").ap()
    out_dram = nc.dram_tensor("outT", [D, T], F32R, kind="ExternalOutput").ap()

    scale = float(DH) ** -0.5

    with tile.TileContext(nc) as tc:
        from contextlib import ExitStack
        with ExitStack() as ctx:
            pc = ctx.enter_context(tc.tile_pool(name="const", bufs=1))
            pres = ctx.enter_context(tc.tile_pool(name="res", bufs=5))
            pn = ctx.enter_context(tc.tile_pool(name="n", bufs=5))
            psq = ctx.enter_context(tc.tile_pool(name="sq", bufs=1))
            prow = ctx.enter_context(tc.tile_pool(name="row", bufs=1))
            pbc = ctx.enter_context(tc.tile_pool(name="bc", bufs=2))
            prcb = ctx.enter_context(tc.tile_pool(name="rcb", bufs=2))
            pw = ctx.enter_context(tc.tile_pool(name="w", bufs=6))
            pps = ctx.enter_context(tc.tile_pool(name="ps", bufs=2, space="PSUM"))

            bias_sb = pc.tile([128, NB], FP32, tag="bias")
            nc.sync.dma_start(bias_sb[:], dram["biases"][:])
            invd_f = pc.tile([128, 1], FP32, tag="invdf")
            nc.vector.memset(invd_f[:], 1.0 / D)
            invd = pc.tile([128, 1], F32R, tag="invd")
            nc.vector.tensor_copy(invd[:], invd_f[:])  # fp32r rounding producer
            onesr_f = pc.tile([128, 128], FP32, tag="onesrf")
            nc.vector.memset(onesr_f[:], 1.0)
            onesr = pc.tile([128, 128], F32R, tag="onesr")
            nc.vector.tensor_copy(onesr[:], onesr_f[:])

            def bcol(j):
                return bias_sb[:, j:j + 1]

            def load_w(dname, n_kt, tag, pool, dtype=F32R):
                tiles = []
                for kt in range(n_kt):
                    wt = pool.tile([128, D], dtype, tag=tag, name=f"{dname}_{kt}")
                    nc.sync.dma_start(wt[:], dram[dname][kt * 128:(kt + 1) * 128, :])
                    tiles.append(wt)
                return tiles

            def emit_ln(x_tiles, which, out_tiles):
                """Feature-major LN of 5 [128, T] fp32r tiles.

                Column stats via fp32r ones-matmuls; mean/rstd rows for the
                two query halves are packed at partitions 0/32 so one batched
                DVE reciprocal serves both, and broadcasting across
                partitions is a PE ones-column outer product into PSUM
                (gpsimd partition_broadcast corrupts offset-row sources on
                HW). out_tiles: list that receives the 5 result APs; passing
                x_tiles itself runs the LN in place."""
                in_place = out_tiles is x_tiles
                mup = prow.tile([128, QH], F32R, tag="mup", bufs=2, name=f"mup{which}")
                msqp = prow.tile([128, QH], FP32, tag="msqp", bufs=2, name=f"msqp{which}")
                rstd = prow.tile([128, QH], F32R, tag="rstd", bufs=2, name=f"rstd{which}")
                mu_b = {}
                for hh in range(2):
                    sl = slice(hh * QH, (hh + 1) * QH)
                    r0 = 32 * hh
                    stp = pps.tile([128, 2 * QH], FP32, tag="sps", bufs=2,
                                   name=f"lnps{which}{hh}")
                    sp = stp[:, 0:QH]
                    spq = stp[:, QH:2 * QH]
                    for kt in range(DKT):
                        nc.tensor.matmul(sp[0:1, :], invd[:, 0:1],
                                         x_tiles[kt][:, sl],
                                         start=(kt == 0), stop=(kt == DKT - 1))
                    for kt in range(DKT):
                        sq = psq.tile([128, QH], F32R, tag="sq", name=f"sq{which}{hh}{kt}")
                        nc.scalar.square(sq[:], x_tiles[kt][:, sl])
                        nc.tensor.matmul(spq[0:1, :], invd[:, 0:1], sq[:],
                                         start=(kt == 0), stop=(kt == DKT - 1))
                    nc.vector.tensor_copy(mup[r0:r0 + 1, :], sp[0:1, :])
                    nc.vector.tensor_copy(msqp[r0:r0 + 1, :], spq[0:1, :])
                    mb = pps.tile([128, QH], FP32, tag="avps", bufs=2,
                                  name=f"mub{which}{hh}")
                    nc.tensor.matmul(mb[:, :], onesr[r0:r0 + 1, :],
                                     mup[r0:r0 + 1, :], start=True, stop=True)
                    mu_b[hh] = mb
                    # pass 1: x - mu (frees the mu broadcast PSUM bank early)
                    for kt in range(DKT):
                        if in_place:
                            nt_seg = x_tiles[kt][:, sl]
                        else:
                            if hh == 0:
                                nt = pn.tile([128, T], F32R, tag="n",
                                             name=f"n{which}_{kt}")
                                out_tiles.append(nt)
                            nt_seg = out_tiles[kt][:, sl]
                        nc.vector.tensor_tensor(nt_seg, x_tiles[kt][:, sl],
                                                mu_b[hh][:, :], OP.subtract)
                    # -var = mu^2 - E[x^2] at the packed row
                    nc.vector.tensor_tensor(mup[r0:r0 + 1, :], mup[r0:r0 + 1, :],
                                            mup[r0:r0 + 1, :], OP.mult)
                    nc.vector.tensor_tensor(mup[r0:r0 + 1, :], mup[r0:r0 + 1, :],
                                            msqp[r0:r0 + 1, :], OP.subtract)
                    # rstd = exp(-0.5 * ln(var + eps)); ACT Ln/Exp round trip
                    # measured at 1.1e-5 max rel on HW, and keeps the whole
                    # tail off the (busier) vector engine
                    nc.scalar.activation(msqp[r0:r0 + 1, :], mup[r0:r0 + 1, :],
                                         AF.Ln, scale=-1.0,
                                         bias=bias_sb[0:1, EPS_COL:EPS_COL + 1])
                    nc.scalar.activation(rstd[r0:r0 + 1, :], msqp[r0:r0 + 1, :],
                                         AF.Exp, scale=-0.5)
                for hh in range(2):
                    sl = slice(hh * QH, (hh + 1) * QH)
                    r0 = 32 * hh
                    rb = pps.tile([128, QH], FP32, tag="avps", bufs=2,
                                  name=f"rb{which}{hh}")
                    nc.tensor.matmul(rb[:, :], onesr[r0:r0 + 1, :],
                                     rstd[r0:r0 + 1, :], start=True, stop=True)
                    for kt in range(DKT):
                        nt_seg = (x_tiles[kt] if in_place else out_tiles[kt])[:, sl]
                        nc.vector.tensor_tensor(nt_seg, nt_seg, rb[:, :], OP.mult)
                        if not ln_trivial[which - 1]:
                            nc.scalar.activation(nt_seg, nt_seg, AF.Identity,
                                                 bias=bcol(LN_B[which] + kt),
                                                 scale=bcol(LN_G[which] + kt))
                return out_tiles

            def head_proj(w_tiles, n_tiles, out_tiles, col_off, n_kt, tag):
                """out^T[h][0:80, col_off:col_off+T] = w.T @ n, per-head padded."""
                for h in range(H):
                    for hh in range(2):
                        qp = pps.tile([128, QH], FP32, tag="ps", name=f"hp{tag}{h}{hh}")
                        for kt in range(n_kt):
                            nc.tensor.matmul(
                                qp[0:DH, :],
                                r32(w_tiles[kt][:, h * DH:(h + 1) * DH]),
                                r32(n_tiles[kt][:, hh * QH:(hh + 1) * QH]),
                                start=(kt == 0), stop=(kt == n_kt - 1))
                        nc.vector.tensor_copy(
                            out_tiles[h][0:DH, col_off + hh * QH:col_off + (hh + 1) * QH],
                            qp[0:DH, :])

            def v_proj(n_tiles, vt, n_kt, w_tiles, n_tok, tok_off):
                """token-major V tile, per-head 97-col slots: data cols 0:80,
                ones col at 96 so the AV denominator lands on PSUM partition
                96 (engine APs must start at partition 0/32/64/96)."""
                pad_ap = vt[:, 0:776].rearrange("p (h c) -> p h c", c=97)[:, :, 80:96]
                nc.vector.memset(pad_ap, 0.0)
                ones_ap = vt[:, 0:776].rearrange("p (h c) -> p h c", c=97)[:, :, 96:97]
                nc.vector.memset(ones_ap, 1.0)
                vpp = pps.tile([128, 2 * QH], FP32, tag="sps", bufs=2, name="vpp")
                for half in range(2):
                    vp = vpp[:, half * QH:half * QH + 320]
                    for kt in range(n_kt):
                        nc.tensor.matmul(
                            vp[0:n_tok, :],
                            r32(n_tiles[kt][:, tok_off:tok_off + n_tok]),
                            r32(w_tiles[kt][:, half * 320:(half + 1) * 320]),
                            start=(kt == 0), stop=(kt == n_kt - 1))
                    dst = vt[:, half * 388:half * 388 + 388].rearrange(
                        "p (h c) -> p h c", c=97)[0:n_tok, :, 0:80]
                    src = vp[0:n_tok, :].rearrange("p (h c) -> p h c", c=80)
                    nc.vector.tensor_copy(dst, src)

            def attention(qT_t, kT_t, v_t, n_keytiles, key_dim_last, aT_t, e_pool):
                """S^T -> exp -> AV; attention output is evicted unnormalized
                and the 16 per-(head, q-half) denominators are batched into
                two 32-row-aligned tiles so just two accurate reciprocals run
                (a [1,512] DVE reciprocal costs ~3.3us; 32 of them dominated
                the v1 profile)."""
                den_t = {}
                denr_t = {}

                def dslot(p):
                    return p // 3, 32 * (p % 3)

                def emit_group_normalize(t):
                    """reciprocal of den tile t + normalize its pairs."""
                    dr = prcb.tile([128, QH], F32R, tag="denr", bufs=3,
                                   name=f"denr{t}")
                    with nc.allow_low_precision(reason="fp32r denom rounding"):
                        nc.vector.reciprocal(dr[:], den_t[t][:])
                    denr_t[t] = dr
                    for p in range(3 * t, min(3 * t + 3, n_pairs)):
                        h, hh = p // 2, p % 2
                        _, drow = dslot(p)
                        rcb = pps.tile([128, QH], FP32, tag="ps", bufs=2,
                                       name=f"rcb{h}{hh}")
                        nc.tensor.matmul(
                            rcb[0:DH, :], onesr[drow:drow + 1, 0:DH],
                            dr[drow:drow + 1, :], start=True, stop=True)
                        seg = aT_t[h][0:DH, hh * QH:(hh + 1) * QH]
                        nc.vector.tensor_tensor(seg, seg, rcb[0:DH, :], OP.mult)
                npairs = (n_keytiles + 1) // 2
                n_pairs = 2 * H
                for h in range(H):
                    at = aT_t[h]
                    # rows 80:128 are padding consumed by the padded out-proj;
                    # zero from 64 (SBUF APs must start at partition 0/32/64/96)
                    nc.vector.memset(at[64:128, :], 0.0)
                    for hh in range(2):
                        p = h * 2 + hh
                        avp = pps.tile([128, QH], FP32, tag="avps", bufs=2,
                                       name=f"av{h}{hh}")
                        # two score tiles share one 2-bank PSUM tile so a
                        # single exp covers both (halves the ACT op count);
                        # pipelined one pair ahead of the AV consumers
                        ets = {}
                        for pt in range(npairs + 1):
                            if pt < npairs:
                                kts = [kt for kt in (2 * pt, 2 * pt + 1)
                                       if kt < n_keytiles]
                                spp = pps.tile([128, 2 * QH], FP32, tag="sps",
                                               bufs=2, name=f"s{h}{hh}{pt}")
                                klens = []
                                for j, kt in enumerate(kts):
                                    klen = (key_dim_last
                                            if kt == n_keytiles - 1 else 128)
                                    klens.append(klen)
                                    nc.tensor.matmul(
                                        spp[0:klen, j * QH:(j + 1) * QH],
                                        kT_t[h][0:DH, kt * 128:kt * 128 + klen],
                                        qT_t[h][0:DH, hh * QH:(hh + 1) * QH],
                                        start=True, stop=True)
                                et = e_pool.tile([128, 2 * QH], BF16, tag="E",
                                                 name=f"e{h}{hh}{pt}")
                                if len(kts) == 2 and klens[0] == klens[1]:
                                    nc.scalar.activation(
                                        et[0:klens[0], :], spp[0:klens[0], :],
                                        AF.Exp, scale=scale)
                                else:
                                    for j, kt in enumerate(kts):
                                        nc.scalar.activation(
                                            et[0:klens[j], j * QH:(j + 1) * QH],
                                            spp[0:klens[j], j * QH:(j + 1) * QH],
                                            AF.Exp, scale=scale)
                                ets[pt] = (et, kts, klens)
                            if pt > 0:
                                pet, pkts, pklens = ets.pop(pt - 1)
                                for j, kt in enumerate(pkts):
                                    nc.tensor.matmul(
                                        avp[0:97, :],
                                        v_t[kt][0:pklens[j], h * 97:(h + 1) * 97],
                                        pet[0:pklens[j], j * QH:(j + 1) * QH],
                                        start=(kt == 0), stop=(kt == n_keytiles - 1))
                        # unnormalized evict (frees the PSUM bank) + denom stash
                        nc.vector.tensor_copy(at[0:DH, hh * QH:(hh + 1) * QH],
                                              avp[0:DH, :])
                        dt_i, drow = dslot(p)
                        if dt_i not in den_t:
                            dn = prcb.tile([128, QH], FP32, tag="den", bufs=3,
                                           name=f"den{dt_i}")
                            nc.vector.memset(dn[:], 1.0)
                            den_t[dt_i] = dn
                        nc.vector.tensor_copy(
                            den_t[dt_i][drow:drow + 1, :], avp[96:97, :])
                        if p == 3 * dt_i + 2 or p == n_pairs - 1:
                            emit_group_normalize(dt_i)


            def out_proj(wp_tiles, aT_t, res_t, bias_off):
                """res += aT @ o^T + bias (in-place residual update)."""
                for m in range(DKT):
                    for hh in range(2):
                        op_ = pps.tile([128, QH], FP32, tag="ps", name=f"op{m}{hh}")
                        for kt in range(H):
                            nc.tensor.matmul(
                                op_[:, :],
                                wp_tiles[kt][:, m * 128:(m + 1) * 128],
                                aT_t[kt][:, hh * QH:(hh + 1) * QH],
                                start=(kt == 0), stop=(kt == H - 1))
                        seg = res_t[m][:, hh * QH:(hh + 1) * QH]
                        nc.vector.scalar_tensor_tensor(
                            seg, op_[:, :], bcol(bias_off + m), seg, OP.add, OP.add)

            # residual stream (feature-major, f32)
            res_tiles = []
            for kt in range(DKT):
                rt = pres.tile([128, T], F32R, tag="res", name=f"res_{kt}")
                nc.sync.dma_start(rt[:], dram["hsT_q"][kt * 128:(kt + 1) * 128, :])
                res_tiles.append(rt)

            with ExitStack() as ctx_abcd:
                pqT = ctx_abcd.enter_context(tc.tile_pool(name="qT", bufs=8))
                paT = ctx_abcd.enter_context(tc.tile_pool(name="aT", bufs=8))

                # ---------- phase A: LN1 + QKV projections ----------
                with ExitStack() as ctx_b:
                    pkT = ctx_b.enter_context(tc.tile_pool(name="kT", bufs=8))
                    pV = ctx_b.enter_context(tc.tile_pool(name="V", bufs=16))
                    pE = ctx_b.enter_context(tc.tile_pool(name="E", bufs=4))

                    kT_tiles = [pkT.tile([128, 2 * T], BF16, tag="kT", name=f"kT_{h}")
                                for h in range(H)]
                    v_tiles = [pV.tile([128, 776], BF16, tag="V", name=f"v_{i}")
                               for i in range(2 * TT)]

                    n_q = emit_ln(res_tiles, 1, [])
                    fr0_tiles = []
                    for kt in range(DKT):
                        ft = pn.tile([128, T], F32R, tag="fr", bufs=5,
                                     name=f"fr0_{kt}")
                        nc.sync.dma_start(
                            ft[:], dram["hsT_first"][kt * 128:(kt + 1) * 128, :])
                        fr0_tiles.append(ft)
                    emit_ln(fr0_tiles, 1, fr0_tiles)  # in place, overlaps Q proj
                    q1_sb = load_w("q1", DKT, "w", pw)
                    qT_tiles = [pqT.tile([128, T], BF16, tag="qT", name=f"qT_{h}")
                                for h in range(H)]
                    head_proj(q1_sb, n_q, qT_tiles, 0, DKT, "q")

                    for fi, fr_tiles in enumerate((fr0_tiles, None)):
                        if fr_tiles is None:
                            fr_tiles = []
                            for kt in range(DKT):
                                ft = pn.tile([128, T], F32R, tag="fr", bufs=5,
                                             name=f"fr1_{kt}")
                                nc.sync.dma_start(
                                    ft[:],
                                    dram["hsT_former"][kt * 128:(kt + 1) * 128, :])
                                fr_tiles.append(ft)
                            emit_ln(fr_tiles, 1, fr_tiles)  # in place
                        k1_sb = load_w("k1", DKT, "w", pw)
                        head_proj(k1_sb, fr_tiles, kT_tiles, fi * T, DKT, f"k{fi}")
                        v1_sb = load_w("v1", DKT, "w", pw)
                        for tt in range(TT):
                            v_proj(fr_tiles, v_tiles[fi * TT + tt], DKT, v1_sb,
                                   128, tt * 128)

                    # ---------- phase B: sparse-causal attention ----------
                    aT_tiles = [paT.tile([128, T], BF16, tag="aT", name=f"aT_{h}")
                                for h in range(H)]
                    attention(qT_tiles, kT_tiles, v_tiles, 2 * TT, 128, aT_tiles, pE)

                # ---------- phase C: o1 + residual ----------
                with ExitStack() as ctx_c:
                    pwp = ctx_c.enter_context(tc.tile_pool(name="wp", bufs=8))
                    o1p_sb = load_w("o1p", H, "wp", pwp, dtype=BF16)
                    out_proj(o1p_sb, aT_tiles, res_tiles, OB1)

                # ---------- phase D: cross attention ----------
                with ExitStack() as ctx_d:
                    penc = ctx_d.enter_context(tc.tile_pool(name="enc", bufs=6))
                    pk2 = ctx_d.enter_context(tc.tile_pool(name="k2T", bufs=8))
                    pV2 = ctx_d.enter_context(tc.tile_pool(name="V2", bufs=1))
                    pE2 = ctx_d.enter_context(tc.tile_pool(name="E2", bufs=4))
                    pwp2 = ctx_d.enter_context(tc.tile_pool(name="wp2", bufs=8))

                    n2 = emit_ln(res_tiles, 2, [])
                    q2_sb = load_w("q2", DKT, "w", pw)
                    q2T_tiles = [pqT.tile([128, T], BF16, tag="qT", name=f"q2T_{h}")
                                 for h in range(H)]
                    head_proj(q2_sb, n2, q2T_tiles, 0, DKT, "q2")

                    enc_tiles = []
                    for kt in range(CKT):
                        et_ = penc.tile([128, CTXP], F32R, tag="enc", name=f"enc_{kt}")
                        nc.sync.dma_start(
                            et_[:], dram["encT"][kt * 128:(kt + 1) * 128, :])
                        enc_tiles.append(et_)
                    k2_sb = load_w("k2", CKT, "w", pw)
                    k2T_tiles = [pk2.tile([128, CTXP], BF16, tag="k2T", name=f"k2T_{h}")
                                 for h in range(H)]
                    for h in range(H):
                        kp = pps.tile([128, CTXP], FP32, tag="ps", name=f"k2p{h}")
                        for kt in range(CKT):
                            nc.tensor.matmul(kp[0:DH, :],
                                             r32(k2_sb[kt][:, h * DH:(h + 1) * DH]),
                                             r32(enc_tiles[kt][:]),
                                             start=(kt == 0), stop=(kt == CKT - 1))
                        nc.vector.tensor_copy(k2T_tiles[h][0:DH, :], kp[0:DH, :])
                    v2_sb = load_w("v2", CKT, "w", pw)
                    v2_t = pV2.tile([128, 776], BF16, tag="V2", name="v2t")
                    v_proj(enc_tiles, v2_t, CKT, v2_sb, CTX, 0)

                    a2T_tiles = [paT.tile([128, T], BF16, tag="aT", name=f"a2T_{h}")
                                 for h in range(H)]
                    attention(q2T_tiles, k2T_tiles, [v2_t], 1, CTX, a2T_tiles, pE2)
                    o2p_sb = load_w("o2p", H, "wp2", pwp2, dtype=BF16)
                    out_proj(o2p_sb, a2T_tiles, res_tiles, OB2)

            # ---------- phase E: GEGLU feed-forward ----------
            with ExitStack() as ctx_e:
                pG = ctx_e.enter_context(tc.tile_pool(name="gT", bufs=20))
                pgl = ctx_e.enter_context(tc.tile_pool(name="gl", bufs=3))
                pff2 = ctx_e.enter_context(tc.tile_pool(name="ff2w", bufs=20))

                n3 = emit_ln(res_tiles, 3, [])
                gT_tiles = []
                for mi in range(FMT):
                    fx = pw.tile([128, D], F32R, tag="w", name=f"fx{mi}")
                    fg = pw.tile([128, D], F32R, tag="w", name=f"fg{mi}")
                    fx_dst = fx[:].rearrange("p (k c) -> p k c", c=128)
                    fg_dst = fg[:].rearrange("p (k c) -> p k c", c=128)
                    fx_src = dram["ff1b"][mi].rearrange("(k p) c -> p k c", p=128)
                    fg_src = dram["ff1b"][FMT + mi].rearrange("(k p) c -> p k c", p=128)
                    nc.sync.dma_start(fx_dst, fx_src)
                    nc.sync.dma_start(fg_dst, fg_src)
                    gt = pG.tile([128, T], BF16, tag="gT", name=f"gT_{mi}")
                    gT_tiles.append(gt)
                    for hh in range(2):
                        xgp = pps.tile([128, 2 * QH], FP32, tag="sps", bufs=2,
                                       name=f"xgp{mi}{hh}")
                        xp = xgp[:, 0:QH]
                        gp = xgp[:, QH:2 * QH]
                        for kt in range(DKT):
                            nc.tensor.matmul(
                                xp[:, :], r32(fx[:, kt * 128:(kt + 1) * 128]),
                                r32(n3[kt][:, hh * QH:(hh + 1) * QH]),
                                start=(kt == 0), stop=(kt == DKT - 1))
                        for kt in range(DKT):
                            nc.tensor.matmul(
                                gp[:, :], r32(fg[:, kt * 128:(kt + 1) * 128]),
                                r32(n3[kt][:, hh * QH:(hh + 1) * QH]),
                                start=(kt == 0), stop=(kt == DKT - 1))
                        gl = pgl.tile([128, QH], BF16, tag="gl", name=f"gl{mi}{hh}")
                        nc.scalar.activation(gl[:], gp[:, :], GELU_AF or AF.Gelu,
                                             bias=bcol(FBG + mi), scale=1.0)
                        nc.vector.scalar_tensor_tensor(
                            gt[:, hh * QH:(hh + 1) * QH], xp[:, :], bcol(FBX + mi),
                            gl[:], OP.add, OP.mult)

                ff2_sb = load_w("ff2", FMT, "ff2w", pff2, dtype=BF16)
                for m in range(DKT):
                    for hh in range(2):
                        fp = pps.tile([128, QH], FP32, tag="ps", name=f"fp{m}{hh}")
                        for kt in range(FMT):
                            nc.tensor.matmul(
                                fp[:, :], ff2_sb[kt][:, m * 128:(m + 1) * 128],
                                gT_tiles[kt][:, hh * QH:(hh + 1) * QH],
                                start=(kt == 0), stop=(kt == FMT - 1))
                        seg = res_tiles[m][:, hh * QH:(hh + 1) * QH]
                        nc.vector.scalar_tensor_tensor(
                            seg, fp[:, :], bcol(FB2 + m), seg, OP.add, OP.add)
            for m in range(DKT):
                nc.sync.dma_start(out_dram[m * 128:(m + 1) * 128, :], res_tiles[m][:])

    nc.compile()
    return nc


_PROGRAM_CACHE = {}


def _get_program(ln_trivial):
    key = (tuple(ln_trivial), GELU_AF)
    if key not in _PROGRAM_CACHE:
        _PROGRAM_CACHE[key] = build_program(ln_trivial)
    return _PROGRAM_CACHE[key]


def _pad_heads(w):
    """[640, 640] head rows -> [1024, 640] padded to 128/head."""
    out = np.zeros((H * 128, D), np.float32)
    for h in range(H):
        out[h * 128:h * 128 + DH] = w[h * DH:(h + 1) * DH]
    return out


def _bias_cols(vec, n):
    return np.ascontiguousarray(vec.reshape(n, 128).T)


def kernel(**inputs):
    hs = np.ascontiguousarray(inputs["hidden_states"], np.float32)
    enc = np.ascontiguousarray(inputs["encoder_hidden_states"], np.float32)
    f = int(inputs["video_length"])
    BF = hs.shape[0]
    assert BF == N_CORES and hs.shape[1:] == (T, D)

    ln_trivial = tuple(
        bool(np.all(inputs[f"n{i}_g"] == 1.0) and np.all(inputs[f"n{i}_b"] == 0.0))
        for i in (1, 2, 3))
    nc = _get_program(ln_trivial)

    biases = np.zeros((128, NB), np.float32)
    biases[:, EPS_COL] = LN_EPS
    biases[:, OB1:OB1 + 5] = _bias_cols(inputs["o1_b"].astype(np.float32), 5)
    biases[:, OB2:OB2 + 5] = _bias_cols(inputs["o2_b"].astype(np.float32), 5)
    biases[:, FB2:FB2 + 5] = _bias_cols(inputs["ff2_b"].astype(np.float32), 5)
    ff1_b = inputs["ff1_b"].astype(np.float32)
    biases[:, FBX:FBX + FMT] = _bias_cols(ff1_b[:DFF], FMT)
    biases[:, FBG:FBG + FMT] = _bias_cols(ff1_b[DFF:], FMT)
    for i in (1, 2, 3):
        biases[:, LN_G[i]:LN_G[i] + 5] = _bias_cols(inputs[f"n{i}_g"].astype(np.float32), 5)
        biases[:, LN_B[i]:LN_B[i] + 5] = _bias_cols(inputs[f"n{i}_b"].astype(np.float32), 5)

    ff1 = inputs["ff1"].astype(np.float32)  # [640, 5120]
    ff1b = np.ascontiguousarray(
        ff1.reshape(DKT, 128, 2 * FMT, 128).transpose(2, 0, 1, 3).reshape(2 * FMT, D, 128))

    common = {
        "q1": np.ascontiguousarray(inputs["q1"], np.float32),
        "k1": np.ascontiguousarray(inputs["k1"], np.float32),
        "v1": np.ascontiguousarray(inputs["v1"], np.float32),
        "q2": np.ascontiguousarray(inputs["q2"], np.float32),
        "k2": np.ascontiguousarray(inputs["k2"], np.float32),
        "v2": np.ascontiguousarray(inputs["v2"], np.float32),
        "o1p": _pad_heads(inputs["o1"].astype(np.float32)).astype(ml_dtypes.bfloat16),
        "o2p": _pad_heads(inputs["o2"].astype(np.float32)).astype(ml_dtypes.bfloat16),
        "ff1b": ff1b,
        "ff2": np.ascontiguousarray(inputs["ff2"], np.float32).astype(ml_dtypes.bfloat16),
        "biases": biases,
    }

    hsT = np.ascontiguousarray(hs.transpose(0, 2, 1))      # [BF, 640, 1024]
    encT = np.zeros((BF, CROSS, CTXP), np.float32)         # ctx padded 77 -> 80
    encT[:, :, :CTX] = enc.transpose(0, 2, 1)
    in_maps = []
    for g in range(BF):
        bi, fi = divmod(g, f)
        first = bi * f
        former = bi * f + max(fi - 1, 0)
        in_maps.append({
            **common,
            "hsT_q": hsT[g],
            "hsT_first": hsT[first],
            "hsT_former": hsT[former],
            "encT": encT[g],
        })

    res = run_bass_kernel_spmd(nc, in_maps, core_ids=list(range(N_CORES)),
                               trace=bool(int(os.environ.get("KERNEL_TRACE", "0"))))
    kernel.last_results = res
    out = np.stack([res.results[g]["outT"].T for g in range(BF)])
    return np.ascontiguousarray(out.astype(inputs["hidden_states"].dtype))


# revision 32
# speedup vs baseline: 1.0716x; 1.0176x over previous
"""Trainium2 Bass kernel for a video-diffusion BasicTransformerBlock
(sparse-causal self-attn + cross-attn + GEGLU FF).

Sharding: data-parallel, one (batch, frame) pair per NeuronCore (8 frames ->
8 cores). Each core receives its own frame, frame 0 of its batch, and the
previous frame (duplicated inputs), so the sparse-causal KV gather needs no
collectives. For frames 0/1 the first/former KV frames coincide; softmax over
duplicated keys is mathematically identical to the reference's concat.

On-device layout: activations are feature-major (x^T, [dim, tokens]) so every
projection contracts over SBUF partitions without any transposes. LayerNorm
column-stats come from ones-matmuls; softmax runs max-free (scores are
bounded ~|5.5|) with denominators from an appended ones-column in V.
All transposes happen host-side in numpy.
"""
import os
import sys
import numpy as np

if not os.environ.get("TRN_TERMINAL_POOL_IPS"):
    raise RuntimeError("expected axon trn environment")
for _p in ("/opt/trn_rl_repo",):
    if _p not in sys.path:
        sys.path.append(_p)

import ml_dtypes
import concourse.bass as bass
import concourse.tile as tile
from concourse import bacc, mybir
from concourse.bass_utils import run_bass_kernel_spmd

FP32 = mybir.dt.float32
F32R = mybir.dt.float32r
BF16 = mybir.dt.bfloat16
AF = mybir.ActivationFunctionType
OP = mybir.AluOpType

D = 640          # model dim
T = 1024         # tokens / frame
H = 8            # heads
DH = 80          # head dim
DKT = D // 128   # 5 feature tiles of the model dim
TT = T // 128    # 8 token tiles / frame
QH = 512         # query half width
CROSS = 768
CKT = CROSS // 128
CTX = 77
CTXP = 80   # context padded for fp32r free-dim alignment
DFF = 2560       # ff hidden (per GEGLU half)
FMT = DFF // 128  # 20 ff row tiles per half
LN_EPS = 1e-5

# bias-pack column offsets ([128, NB] f32)
OB1, OB2, FB2, FBX, FBG = 0, 5, 10, 15, 35
LN_G = {1: 55, 2: 65, 3: 75}
LN_B = {1: 60, 2: 70, 3: 80}
EPS_COL = 85
NB = 86

N_CORES = 8

# test hook: CoreSim lacks Gelu; tests may override with a sim-supported func
GELU_AF = None


def r32(ap):
    return ap if ap.dtype == F32R else ap.bitcast(F32R)


def build_program(ln_trivial):
    nc = bacc.Bacc("TRN2", target_bir_lowering=False, debug=False,
                   num_devices=N_CORES)
    dram = {}
    for name in ("hsT_q", "hsT_first", "hsT_former"):
        dram[name] = nc.dram_tensor(name, [D, T], F32R, kind="ExternalInput").ap()
    dram["encT"] = nc.dram_tensor("encT", [CROSS, CTXP], F32R, kind="ExternalInput").ap()
    for name in ("q1", "k1", "v1", "q2"):
        dram[name] = nc.dram_tensor(name, [D, D], F32R, kind="ExternalInput").ap()
    for name in ("k2", "v2"):
        dram[name] = nc.dram_tensor(name, [CROSS, D], F32R, kind="ExternalInput").ap()
    for name in ("o1p", "o2p"):
        dram[name] = nc.dram_tensor(name, [H * 128, D], BF16, kind="ExternalInput").ap()
    dram["ff1b"] = nc.dram_tensor("ff1b", [2 * FMT, D, 128], F32R, kind="ExternalInput").ap()
    dram["ff2"] = nc.dram_tensor("ff2", [DFF, D], BF16, kind="ExternalInput").ap()
    dram["biases"] = nc.dram_tensor("biases", [128, NB], FP32, kind="ExternalInput").ap()
    out_dram = nc.dram_tensor("outT", [D, T], F32R, kind="ExternalOutput").ap()

    scale = float(DH) ** -0.5

    with tile.TileContext(nc) as tc:
        from contextlib import ExitStack
        with ExitStack() as ctx:
            pc = ctx.enter_context(tc.tile_pool(name="const", bufs=1))
            pres = ctx.enter_context(tc.tile_pool(name="res", bufs=5))
            pn = ctx.enter_context(tc.tile_pool(name="n", bufs=5))
            psq = ctx.enter_context(tc.tile_pool(name="sq", bufs=1))
            prow = ctx.enter_context(tc.tile_pool(name="row", bufs=1))
            pbc = ctx.enter_context(tc.tile_pool(name="bc", bufs=2))
            prcb = ctx.enter_context(tc.tile_pool(name="rcb", bufs=2))
            pw = ctx.enter_context(tc.tile_pool(name="w", bufs=6))
            pps = ctx.enter_context(tc.tile_pool(name="ps", bufs=2, space="PSUM"))

            bias_sb = pc.tile([128, NB], FP32, tag="bias")
            nc.sync.dma_start(bias_sb[:], dram["biases"][:])
            invd_f = pc.tile([128, 1], FP32, tag="invdf")
            nc.vector.memset(invd_f[:], 1.0 / D)
            invd = pc.tile([128, 1], F32R, tag="invd")
            nc.vector.tensor_copy(invd[:], invd_f[:])  # fp32r rounding producer
            onesr_f = pc.tile([128, 128], FP32, tag="onesrf")
            nc.vector.memset(onesr_f[:], 1.0)
            onesr = pc.tile([128, 128], F32R, tag="onesr")
            nc.vector.tensor_copy(onesr[:], onesr_f[:])

            def bcol(j):
                return bias_sb[:, j:j + 1]

            def load_w(dname, n_kt, tag, pool, dtype=F32R):
                tiles = []
                for kt in range(n_kt):
                    wt = pool.tile([128, D], dtype, tag=tag, name=f"{dname}_{kt}")
                    nc.sync.dma_start(wt[:], dram[dname][kt * 128:(kt + 1) * 128, :])
                    tiles.append(wt)
                return tiles

            def emit_ln(x_tiles, which, out_tiles):
                """Feature-major LN of 5 [128, T] fp32r tiles.

                Column stats via fp32r ones-matmuls; mean/rstd rows for the
                two query halves are packed at partitions 0/32 so one batched
                DVE reciprocal serves both, and broadcasting across
                partitions is a PE ones-column outer product into PSUM
                (gpsimd partition_broadcast corrupts offset-row sources on
                HW). out_tiles: list that receives the 5 result APs; passing
                x_tiles itself runs the LN in place."""
                in_place = out_tiles is x_tiles
                mup = prow.tile([128, QH], F32R, tag="mup", bufs=2, name=f"mup{which}")
                msqp = prow.tile([128, QH], FP32, tag="msqp", bufs=2, name=f"msqp{which}")
                rstd = prow.tile([128, QH], F32R, tag="rstd", bufs=2, name=f"rstd{which}")
                mu_b = {}
                for hh in range(2):
                    sl = slice(hh * QH, (hh + 1) * QH)
                    r0 = 32 * hh
                    stp = pps.tile([128, 2 * QH], FP32, tag="sps", bufs=2,
                                   name=f"lnps{which}{hh}")
                    sp = stp[:, 0:QH]
                    spq = stp[:, QH:2 * QH]
                    for kt in range(DKT):
                        nc.tensor.matmul(sp[0:1, :], invd[:, 0:1],
                                         x_tiles[kt][:, sl],
                                         start=(kt == 0), stop=(kt == DKT - 1))
                    for kt in range(DKT):
                        sq = psq.tile([128, QH], F32R, tag="sq", name=f"sq{which}{hh}{kt}")
                        nc.scalar.square(sq[:], x_tiles[kt][:, sl])
                        nc.tensor.matmul(spq[0:1, :], invd[:, 0:1], sq[:],
                                         start=(kt == 0), stop=(kt == DKT - 1))
                    nc.vector.tensor_copy(mup[r0:r0 + 1, :], sp[0:1, :])
                    nc.vector.tensor_copy(msqp[r0:r0 + 1, :], spq[0:1, :])
                    mb = pps.tile([128, QH], FP32, tag="avps", bufs=2,
                                  name=f"mub{which}{hh}")
                    nc.tensor.matmul(mb[:, :], onesr[r0:r0 + 1, :],
                                     mup[r0:r0 + 1, :], start=True, stop=True)
                    mu_b[hh] = mb
                    # pass 1: x - mu (frees the mu broadcast PSUM bank early)
                    for kt in range(DKT):
                        if in_place:
                            nt_seg = x_tiles[kt][:, sl]
                        else:
                            if hh == 0:
                                nt = pn.tile([128, T], F32R, tag="n",
                                             name=f"n{which}_{kt}")
                                out_tiles.append(nt)
                            nt_seg = out_tiles[kt][:, sl]
                        nc.vector.tensor_tensor(nt_seg, x_tiles[kt][:, sl],
                                                mu_b[hh][:, :], OP.subtract)
                    # -var = mu^2 - E[x^2] at the packed row
                    nc.vector.tensor_tensor(mup[r0:r0 + 1, :], mup[r0:r0 + 1, :],
                                            mup[r0:r0 + 1, :], OP.mult)
                    nc.vector.tensor_tensor(mup[r0:r0 + 1, :], mup[r0:r0 + 1, :],
                                            msqp[r0:r0 + 1, :], OP.subtract)
                    # rstd = exp(-0.5 * ln(var + eps)); ACT Ln/Exp round trip
                    # measured at 1.1e-5 max rel on HW, and keeps the whole
                    # tail off the (busier) vector engine
                    nc.scalar.activation(msqp[r0:r0 + 1, :], mup[r0:r0 + 1, :],
                                         AF.Ln, scale=-1.0,
                                         bias=bias_sb[0:1, EPS_COL:EPS_COL + 1])
                    nc.scalar.activation(rstd[r0:r0 + 1, :], msqp[r0:r0 + 1, :],
                                         AF.Exp, scale=-0.5)
                for hh in range(2):
                    sl = slice(hh * QH, (hh + 1) * QH)
                    r0 = 32 * hh
                    rb = pps.tile([128, QH], FP32, tag="avps", bufs=2,
                                  name=f"rb{which}{hh}")
                    nc.tensor.matmul(rb[:, :], onesr[r0:r0 + 1, :],
                                     rstd[r0:r0 + 1, :], start=True, stop=True)
                    for kt in range(DKT):
                        nt_seg = (x_tiles[kt] if in_place else out_tiles[kt])[:, sl]
                        nc.vector.tensor_tensor(nt_seg, nt_seg, rb[:, :], OP.mult)
                        if not ln_trivial[which - 1]:
                            nc.scalar.activation(nt_seg, nt_seg, AF.Identity,
                                                 bias=bcol(LN_B[which] + kt),
                                                 scale=bcol(LN_G[which] + kt))
                return out_tiles

            def head_proj(w_tiles, n_tiles, out_tiles, col_off, n_kt, tag):
                """out^T[h][0:80, col_off:col_off+T] = w.T @ n, per-head padded."""
                for h in range(H):
                    for hh in range(2):
                        qp = pps.tile([128, QH], FP32, tag="ps", name=f"hp{tag}{h}{hh}")
                        for kt in range(n_kt):
                            nc.tensor.matmul(
                                qp[0:DH, :],
                                r32(w_tiles[kt][:, h * DH:(h + 1) * DH]),
                                r32(n_tiles[kt][:, hh * QH:(hh + 1) * QH]),
                                start=(kt == 0), stop=(kt == n_kt - 1))
                        nc.vector.tensor_copy(
                            out_tiles[h][0:DH, col_off + hh * QH:col_off + (hh + 1) * QH],
                            qp[0:DH, :])

            def v_proj(n_tiles, vt, n_kt, w_tiles, n_tok, tok_off):
                """token-major V tile, per-head 97-col slots: data cols 0:80,
                ones col at 96 so the AV denominator lands on PSUM partition
                96 (engine APs must start at partition 0/32/64/96)."""
                pad_ap = vt[:, 0:776].rearrange("p (h c) -> p h c", c=97)[:, :, 80:96]
                nc.vector.memset(pad_ap, 0.0)
                ones_ap = vt[:, 0:776].rearrange("p (h c) -> p h c", c=97)[:, :, 96:97]
                nc.vector.memset(ones_ap, 1.0)
                vpp = pps.tile([128, 2 * QH], FP32, tag="sps", bufs=2, name="vpp")
                for half in range(2):
                    vp = vpp[:, half * QH:half * QH + 320]
                    for kt in range(n_kt):
                        nc.tensor.matmul(
                            vp[0:n_tok, :],
                            r32(n_tiles[kt][:, tok_off:tok_off + n_tok]),
                            r32(w_tiles[kt][:, half * 320:(half + 1) * 320]),
                            start=(kt == 0), stop=(kt == n_kt - 1))
                    dst = vt[:, half * 388:half * 388 + 388].rearrange(
                        "p (h c) -> p h c", c=97)[0:n_tok, :, 0:80]
                    src = vp[0:n_tok, :].rearrange("p (h c) -> p h c", c=80)
                    nc.vector.tensor_copy(dst, src)

            def attention(qT_t, kT_t, v_t, n_keytiles, key_dim_last, aT_t, e_pool,
                          recip_on_act=False):
                """S^T -> exp -> AV; attention output is evicted unnormalized
                and the 16 per-(head, q-half) denominators are batched into
                two 32-row-aligned tiles so just two accurate reciprocals run
                (a [1,512] DVE reciprocal costs ~3.3us; 32 of them dominated
                the v1 profile)."""
                den_t = {}
                denr_t = {}

                def dslot(p):
                    return p // 3, 32 * (p % 3)

                def emit_group_normalize(t):
                    """reciprocal of den tile t + normalize its pairs."""
                    dr = prcb.tile([128, QH], F32R, tag="denr", bufs=3,
                                   name=f"denr{t}")
                    if recip_on_act:
                        # 1/x = exp(-ln(x)): ~2e-5 rel, keeps cross-attention
                        # off the vector engine (its PE work is tiny and the
                        # DVE reciprocal would dominate the phase)
                        lt = prcb.tile([128, QH], FP32, tag="denln", bufs=2,
                                       name=f"denln{t}")
                        nc.scalar.activation(lt[:], den_t[t][:], AF.Ln)
                        nc.scalar.activation(dr[:], lt[:], AF.Exp, scale=-1.0)
                    else:
                        with nc.allow_low_precision(reason="fp32r denom rounding"):
                            nc.vector.reciprocal(dr[:], den_t[t][:])
                    denr_t[t] = dr
                    for p in range(3 * t, min(3 * t + 3, n_pairs)):
                        h, hh = p // 2, p % 2
                        _, drow = dslot(p)
                        rcb = pps.tile([128, QH], FP32, tag="ps", bufs=2,
                                       name=f"rcb{h}{hh}")
                        nc.tensor.matmul(
                            rcb[0:DH, :], onesr[drow:drow + 1, 0:DH],
                            dr[drow:drow + 1, :], start=True, stop=True)
                        seg = aT_t[h][0:DH, hh * QH:(hh + 1) * QH]
                        nc.vector.tensor_tensor(seg, seg, rcb[0:DH, :], OP.mult)
                npairs = (n_keytiles + 1) // 2
                n_pairs = 2 * H
                for h in range(H):
                    at = aT_t[h]
                    # rows 80:128 are padding consumed by the padded out-proj;
                    # zero from 64 (SBUF APs must start at partition 0/32/64/96)
                    nc.vector.memset(at[64:128, :], 0.0)
                    for hh in range(2):
                        p = h * 2 + hh
                        avp = pps.tile([128, QH], FP32, tag="avps", bufs=2,
                                       name=f"av{h}{hh}")
                        # two score tiles share one 2-bank PSUM tile so a
                        # single exp covers both (halves the ACT op count);
                        # pipelined one pair ahead of the AV consumers
                        ets = {}
                        for pt in range(npairs + 1):
                            if pt < npairs:
                                kts = [kt for kt in (2 * pt, 2 * pt + 1)
                                       if kt < n_keytiles]
                                spp = pps.tile([128, 2 * QH], FP32, tag="sps",
                                               bufs=2, name=f"s{h}{hh}{pt}")
                                klens = []
                                for j, kt in enumerate(kts):
                                    klen = (key_dim_last
                                            if kt == n_keytiles - 1 else 128)
                                    klens.append(klen)
                                    nc.tensor.matmul(
                                        spp[0:klen, j * QH:(j + 1) * QH],
                                        kT_t[h][0:DH, kt * 128:kt * 128 + klen],
                                        qT_t[h][0:DH, hh * QH:(hh + 1) * QH],
                                        start=True, stop=True)
                                et = e_pool.tile([128, 2 * QH], BF16, tag="E",
                                                 name=f"e{h}{hh}{pt}")
                                if len(kts) == 2 and klens[0] == klens[1]:
                                    nc.scalar.activation(
                                        et[0:klens[0], :], spp[0:klens[0], :],
                                        AF.Exp, scale=scale)
                                else:
                                    for j, kt in enumerate(kts):
                                        nc.scalar.activation(
                                            et[0:klens[j], j * QH:(j + 1) * QH],
                                            spp[0:klens[j], j * QH:(j + 1) * QH],
                                            AF.Exp, scale=scale)
                                ets[pt] = (et, kts, klens)
                            if pt > 0:
                                pet, pkts, pklens = ets.pop(pt - 1)
                                for j, kt in enumerate(pkts):
                                    nc.tensor.matmul(
                                        avp[0:97, :],
                                        v_t[kt][0:pklens[j], h * 97:(h + 1) * 97],
                                        pet[0:pklens[j], j * QH:(j + 1) * QH],
                                        start=(kt == 0), stop=(kt == n_keytiles - 1))
                        # unnormalized evict (frees the PSUM bank) + denom stash
                        nc.vector.tensor_copy(at[0:DH, hh * QH:(hh + 1) * QH],
                                              avp[0:DH, :])
                        dt_i, drow = dslot(p)
                        if dt_i not in den_t:
                            dn = prcb.tile([128, QH], FP32, tag="den", bufs=3,
                                           name=f"den{dt_i}")
                            nc.vector.memset(dn[:], 1.0)
                            den_t[dt_i] = dn
                        nc.vector.tensor_copy(
                            den_t[dt_i][drow:drow + 1, :], avp[96:97, :])
                        if p == 3 * dt_i + 2 or p == n_pairs - 1:
                            emit_group_normalize(dt_i)


            def out_proj(wp_tiles, aT_t, res_t, bias_off):
                """res += aT @ o^T + bias (in-place residual update)."""
                for m in range(DKT):
                    for hh in range(2):
                        op_ = pps.tile([128, QH], FP32, tag="ps", name=f"op{m}{hh}")
                        for kt in range(H):
                            nc.tensor.matmul(
                                op_[:, :],
                                wp_tiles[kt][:, m * 128:(m + 1) * 128],
                                aT_t[kt][:, hh * QH:(hh + 1) * QH],
                                start=(kt == 0), stop=(kt == H - 1))
                        seg = res_t[m][:, hh * QH:(hh + 1) * QH]
                        nc.vector.scalar_tensor_tensor(
                            seg, op_[:, :], bcol(bias_off + m), seg, OP.add, OP.add)

            # residual stream (feature-major, f32)
            res_tiles = []
            for kt in range(DKT):
                rt = pres.tile([128, T], F32R, tag="res", name=f"res_{kt}")
                nc.sync.dma_start(rt[:], dram["hsT_q"][kt * 128:(kt + 1) * 128, :])
                res_tiles.append(rt)

            with ExitStack() as ctx_abcd:
                pqT = ctx_abcd.enter_context(tc.tile_pool(name="qT", bufs=8))
                paT = ctx_abcd.enter_context(tc.tile_pool(name="aT", bufs=8))

                # ---------- phase A: LN1 + QKV projections ----------
                with ExitStack() as ctx_b:
                    pkT = ctx_b.enter_context(tc.tile_pool(name="kT", bufs=8))
                    pV = ctx_b.enter_context(tc.tile_pool(name="V", bufs=16))
                    pE = ctx_b.enter_context(tc.tile_pool(name="E", bufs=4))

                    kT_tiles = [pkT.tile([128, 2 * T], BF16, tag="kT", name=f"kT_{h}")
                                for h in range(H)]
                    v_tiles = [pV.tile([128, 776], BF16, tag="V", name=f"v_{i}")
                               for i in range(2 * TT)]

                    n_q = emit_ln(res_tiles, 1, [])
                    fr0_tiles = []
                    for kt in range(DKT):
                        ft = pn.tile([128, T], F32R, tag="fr", bufs=5,
                                     name=f"fr0_{kt}")
                        nc.sync.dma_start(
                            ft[:], dram["hsT_first"][kt * 128:(kt + 1) * 128, :])
                        fr0_tiles.append(ft)
                    emit_ln(fr0_tiles, 1, fr0_tiles)  # in place, overlaps Q proj
                    q1_sb = load_w("q1", DKT, "w", pw)
                    qT_tiles = [pqT.tile([128, T], BF16, tag="qT", name=f"qT_{h}")
                                for h in range(H)]
                    head_proj(q1_sb, n_q, qT_tiles, 0, DKT, "q")

                    for fi, fr_tiles in enumerate((fr0_tiles, None)):
                        if fr_tiles is None:
                            fr_tiles = []
                            for kt in range(DKT):
                                ft = pn.tile([128, T], F32R, tag="fr", bufs=5,
                                             name=f"fr1_{kt}")
                                nc.sync.dma_start(
                                    ft[:],
                                    dram["hsT_former"][kt * 128:(kt + 1) * 128, :])
                                fr_tiles.append(ft)
                            emit_ln(fr_tiles, 1, fr_tiles)  # in place
                        k1_sb = load_w("k1", DKT, "w", pw)
                        head_proj(k1_sb, fr_tiles, kT_tiles, fi * T, DKT, f"k{fi}")
                        v1_sb = load_w("v1", DKT, "w", pw)
                        for tt in range(TT):
                            v_proj(fr_tiles, v_tiles[fi * TT + tt], DKT, v1_sb,
                                   128, tt * 128)

                    # ---------- phase B: sparse-causal attention ----------
                    aT_tiles = [paT.tile([128, T], BF16, tag="aT", name=f"aT_{h}")
                                for h in range(H)]
                    attention(qT_tiles, kT_tiles, v_tiles, 2 * TT, 128, aT_tiles, pE)

                # ---------- phase C: o1 + residual ----------
                with ExitStack() as ctx_c:
                    pwp = ctx_c.enter_context(tc.tile_pool(name="wp", bufs=8))
                    o1p_sb = load_w("o1p", H, "wp", pwp, dtype=BF16)
                    out_proj(o1p_sb, aT_tiles, res_tiles, OB1)

                # ---------- phase D: cross attention ----------
                with ExitStack() as ctx_d:
                    penc = ctx_d.enter_context(tc.tile_pool(name="enc", bufs=6))
                    pk2 = ctx_d.enter_context(tc.tile_pool(name="k2T", bufs=8))
                    pV2 = ctx_d.enter_context(tc.tile_pool(name="V2", bufs=1))
                    pE2 = ctx_d.enter_context(tc.tile_pool(name="E2", bufs=4))
                    pwp2 = ctx_d.enter_context(tc.tile_pool(name="wp2", bufs=8))

                    n2 = emit_ln(res_tiles, 2, [])
                    q2_sb = load_w("q2", DKT, "w", pw)
                    q2T_tiles = [pqT.tile([128, T], BF16, tag="qT", name=f"q2T_{h}")
                                 for h in range(H)]
                    head_proj(q2_sb, n2, q2T_tiles, 0, DKT, "q2")

                    enc_tiles = []
                    for kt in range(CKT):
                        et_ = penc.tile([128, CTXP], F32R, tag="enc", name=f"enc_{kt}")
                        nc.sync.dma_start(
                            et_[:], dram["encT"][kt * 128:(kt + 1) * 128, :])
                        enc_tiles.append(et_)
                    k2_sb = load_w("k2", CKT, "w", pw)
                    k2T_tiles = [pk2.tile([128, CTXP], BF16, tag="k2T", name=f"k2T_{h}")
                                 for h in range(H)]
                    for h in range(H):
                        kp = pps.tile([128, CTXP], FP32, tag="ps", name=f"k2p{h}")
                        for kt in range(CKT):
                            nc.tensor.matmul(kp[0:DH, :],
                                             r32(k2_sb[kt][:, h * DH:(h + 1) * DH]),
                                             r32(enc_tiles[kt][:]),
                                             start=(kt == 0), stop=(kt == CKT - 1))
                        nc.vector.tensor_copy(k2T_tiles[h][0:DH, :], kp[0:DH, :])
                    v2_sb = load_w("v2", CKT, "w", pw)
                    v2_t = pV2.tile([128, 776], BF16, tag="V2", name="v2t")
                    v_proj(enc_tiles, v2_t, CKT, v2_sb, CTX, 0)

                    a2T_tiles = [paT.tile([128, T], BF16, tag="aT", name=f"a2T_{h}")
                                 for h in range(H)]
                    attention(q2T_tiles, k2T_tiles, [v2_t], 1, CTX, a2T_tiles, pE2,
                              recip_on_act=True)
                    o2p_sb = load_w("o2p", H, "wp2", pwp2, dtype=BF16)
                    out_proj(o2p_sb, a2T_tiles, res_tiles, OB2)

            # ---------- phase E: GEGLU feed-forward ----------
            with ExitStack() as ctx_e:
                pG = ctx_e.enter_context(tc.tile_pool(name="gT", bufs=20))
                pgl = ctx_e.enter_context(tc.tile_pool(name="gl", bufs=3))
                pff2 = ctx_e.enter_context(tc.tile_pool(name="ff2w", bufs=20))

                n3 = emit_ln(res_tiles, 3, [])
                gT_tiles = []
                for mi in range(FMT):
                    fx = pw.tile([128, D], F32R, tag="w", name=f"fx{mi}")
                    fg = pw.tile([128, D], F32R, tag="w", name=f"fg{mi}")
                    fx_dst = fx[:].rearrange("p (k c) -> p k c", c=128)
                    fg_dst = fg[:].rearrange("p (k c) -> p k c", c=128)
                    fx_src = dram["ff1b"][mi].rearrange("(k p) c -> p k c", p=128)
                    fg_src = dram["ff1b"][FMT + mi].rearrange("(k p) c -> p k c", p=128)
                    nc.sync.dma_start(fx_dst, fx_src)
                    nc.sync.dma_start(fg_dst, fg_src)
                    gt = pG.tile([128, T], BF16, tag="gT", name=f"gT_{mi}")
                    gT_tiles.append(gt)
                    for hh in range(2):
                        xgp = pps.tile([128, 2 * QH], FP32, tag="sps", bufs=2,
                                       name=f"xgp{mi}{hh}")
                        xp = xgp[:, 0:QH]
                        gp = xgp[:, QH:2 * QH]
                        for kt in range(DKT):
                            nc.tensor.matmul(
                                xp[:, :], r32(fx[:, kt * 128:(kt + 1) * 128]),
                                r32(n3[kt][:, hh * QH:(hh + 1) * QH]),
                                start=(kt == 0), stop=(kt == DKT - 1))
                        for kt in range(DKT):
                            nc.tensor.matmul(
                                gp[:, :], r32(fg[:, kt * 128:(kt + 1) * 128]),
                                r32(n3[kt][:, hh * QH:(hh + 1) * QH]),
                                start=(kt == 0), stop=(kt == DKT - 1))
                        gl = pgl.tile([128, QH], BF16, tag="gl", name=f"gl{mi}{hh}")
                        nc.scalar.activation(gl[:], gp[:, :], GELU_AF or AF.Gelu,
                                             bias=bcol(FBG + mi), scale=1.0)
                        nc.vector.scalar_tensor_tensor(
                            gt[:, hh * QH:(hh + 1) * QH], xp[:, :], bcol(FBX + mi),
                            gl[:], OP.add, OP.mult)

                ff2_sb = load_w("ff2", FMT, "ff2w", pff2, dtype=BF16)
                for m in range(DKT):
                    for hh in range(2):
                        fp = pps.tile([128, QH], FP32, tag="ps", name=f"fp{m}{hh}")
                        for kt in range(FMT):
                            nc.tensor.matmul(
                                fp[:, :], ff2_sb[kt][:, m * 128:(m + 1) * 128],
                                gT_tiles[kt][:, hh * QH:(hh + 1) * QH],
                                start=(kt == 0), stop=(kt == FMT - 1))
                        seg = res_tiles[m][:, hh * QH:(hh + 1) * QH]
                        nc.vector.scalar_tensor_tensor(
                            seg, fp[:, :], bcol(FB2 + m), seg, OP.add, OP.add)
            for m in range(DKT):
                nc.sync.dma_start(out_dram[m * 128:(m + 1) * 128, :], res_tiles[m][:])

    nc.compile()
    return nc


_PROGRAM_CACHE = {}


def _get_program(ln_trivial):
    key = (tuple(ln_trivial), GELU_AF)
    if key not in _PROGRAM_CACHE:
        _PROGRAM_CACHE[key] = build_program(ln_trivial)
    return _PROGRAM_CACHE[key]


def _pad_heads(w):
    """[640, 640] head rows -> [1024, 640] padded to 128/head."""
    out = np.zeros((H * 128, D), np.float32)
    for h in range(H):
        out[h * 128:h * 128 + DH] = w[h * DH:(h + 1) * DH]
    return out


def _bias_cols(vec, n):
    return np.ascontiguousarray(vec.reshape(n, 128).T)


def kernel(**inputs):
    hs = np.ascontiguousarray(inputs["hidden_states"], np.float32)
    enc = np.ascontiguousarray(inputs["encoder_hidden_states"], np.float32)
    f = int(inputs["video_length"])
    BF = hs.shape[0]
    assert BF == N_CORES and hs.shape[1:] == (T, D)

    ln_trivial = tuple(
        bool(np.all(inputs[f"n{i}_g"] == 1.0) and np.all(inputs[f"n{i}_b"] == 0.0))
        for i in (1, 2, 3))
    nc = _get_program(ln_trivial)

    biases = np.zeros((128, NB), np.float32)
    biases[:, EPS_COL] = LN_EPS
    biases[:, OB1:OB1 + 5] = _bias_cols(inputs["o1_b"].astype(np.float32), 5)
    biases[:, OB2:OB2 + 5] = _bias_cols(inputs["o2_b"].astype(np.float32), 5)
    biases[:, FB2:FB2 + 5] = _bias_cols(inputs["ff2_b"].astype(np.float32), 5)
    ff1_b = inputs["ff1_b"].astype(np.float32)
    biases[:, FBX:FBX + FMT] = _bias_cols(ff1_b[:DFF], FMT)
    biases[:, FBG:FBG + FMT] = _bias_cols(ff1_b[DFF:], FMT)
    for i in (1, 2, 3):
        biases[:, LN_G[i]:LN_G[i] + 5] = _bias_cols(inputs[f"n{i}_g"].astype(np.float32), 5)
        biases[:, LN_B[i]:LN_B[i] + 5] = _bias_cols(inputs[f"n{i}_b"].astype(np.float32), 5)

    ff1 = inputs["ff1"].astype(np.float32)  # [640, 5120]
    ff1b = np.ascontiguousarray(
        ff1.reshape(DKT, 128, 2 * FMT, 128).transpose(2, 0, 1, 3).reshape(2 * FMT, D, 128))

    common = {
        "q1": np.ascontiguousarray(inputs["q1"], np.float32),
        "k1": np.ascontiguousarray(inputs["k1"], np.float32),
        "v1": np.ascontiguousarray(inputs["v1"], np.float32),
        "q2": np.ascontiguousarray(inputs["q2"], np.float32),
        "k2": np.ascontiguousarray(inputs["k2"], np.float32),
        "v2": np.ascontiguousarray(inputs["v2"], np.float32),
        "o1p": _pad_heads(inputs["o1"].astype(np.float32)).astype(ml_dtypes.bfloat16),
        "o2p": _pad_heads(inputs["o2"].astype(np.float32)).astype(ml_dtypes.bfloat16),
        "ff1b": ff1b,
        "ff2": np.ascontiguousarray(inputs["ff2"], np.float32).astype(ml_dtypes.bfloat16),
        "biases": biases,
    }

    hsT = np.ascontiguousarray(hs.transpose(0, 2, 1))      # [BF, 640, 1024]
    encT = np.zeros((BF, CROSS, CTXP), np.float32)         # ctx padded 77 -> 80
    encT[:, :, :CTX] = enc.transpose(0, 2, 1)
    in_maps = []
    for g in range(BF):
        bi, fi = divmod(g, f)
        first = bi * f
        former = bi * f + max(fi - 1, 0)
        in_maps.append({
            **common,
            "hsT_q": hsT[g],
            "hsT_first": hsT[first],
            "hsT_former": hsT[former],
            "encT": encT[g],
        })

    res = run_bass_kernel_spmd(nc, in_maps, core_ids=list(range(N_CORES)),
                               trace=bool(int(os.environ.get("KERNEL_TRACE", "0"))))
    kernel.last_results = res
    out = np.stack([res.results[g]["outT"].T for g in range(BF)])
    return np.ascontiguousarray(out.astype(inputs["hidden_states"].dtype))


# revision 33
# speedup vs baseline: 1.0784x; 1.0063x over previous
"""Trainium2 Bass kernel for a video-diffusion BasicTransformerBlock
(sparse-causal self-attn + cross-attn + GEGLU FF).

Sharding: data-parallel, one (batch, frame) pair per NeuronCore (8 frames ->
8 cores). Each core receives its own frame, frame 0 of its batch, and the
previous frame (duplicated inputs), so the sparse-causal KV gather needs no
collectives. For frames 0/1 the first/former KV frames coincide; softmax over
duplicated keys is mathematically identical to the reference's concat.

On-device layout: activations are feature-major (x^T, [dim, tokens]) so every
projection contracts over SBUF partitions without any transposes. LayerNorm
column-stats come from ones-matmuls; softmax runs max-free (scores are
bounded ~|5.5|) with denominators from an appended ones-column in V.
All transposes happen host-side in numpy.
"""
import os
import sys
import numpy as np

if not os.environ.get("TRN_TERMINAL_POOL_IPS"):
    raise RuntimeError("expected axon trn environment")
for _p in ("/opt/trn_rl_repo",):
    if _p not in sys.path:
        sys.path.append(_p)

import ml_dtypes
import concourse.bass as bass
import concourse.tile as tile
from concourse import bacc, mybir
from concourse.bass_utils import run_bass_kernel_spmd

FP32 = mybir.dt.float32
F32R = mybir.dt.float32r
BF16 = mybir.dt.bfloat16
AF = mybir.ActivationFunctionType
OP = mybir.AluOpType

D = 640          # model dim
T = 1024         # tokens / frame
H = 8            # heads
DH = 80          # head dim
DKT = D // 128   # 5 feature tiles of the model dim
TT = T // 128    # 8 token tiles / frame
QH = 512         # query half width
CROSS = 768
CKT = CROSS // 128
CTX = 77
CTXP = 80   # context padded for fp32r free-dim alignment
DFF = 2560       # ff hidden (per GEGLU half)
FMT = DFF // 128  # 20 ff row tiles per half
LN_EPS = 1e-5

# bias-pack column offsets ([128, NB] f32)
OB1, OB2, FB2, FBX, FBG = 0, 5, 10, 15, 35
LN_G = {1: 55, 2: 65, 3: 75}
LN_B = {1: 60, 2: 70, 3: 80}
EPS_COL = 85
NB = 86

N_CORES = 8

# test hook: CoreSim lacks Gelu; tests may override with a sim-supported func
GELU_AF = None


def r32(ap):
    return ap if ap.dtype == F32R else ap.bitcast(F32R)


def build_program(ln_trivial):
    nc = bacc.Bacc("TRN2", target_bir_lowering=False, debug=False,
                   num_devices=N_CORES)
    dram = {}
    for name in ("hsT_q", "hsT_first", "hsT_former"):
        dram[name] = nc.dram_tensor(name, [D, T], F32R, kind="ExternalInput").ap()
    dram["encT"] = nc.dram_tensor("encT", [CROSS, CTXP], F32R, kind="ExternalInput").ap()
    for name in ("q1", "k1", "v1", "q2"):
        dram[name] = nc.dram_tensor(name, [D, D], F32R, kind="ExternalInput").ap()
    for name in ("k2", "v2"):
        dram[name] = nc.dram_tensor(name, [CROSS, D], F32R, kind="ExternalInput").ap()
    for name in ("o1p", "o2p"):
        dram[name] = nc.dram_tensor(name, [H * 128, D], BF16, kind="ExternalInput").ap()
    dram["ff1b"] = nc.dram_tensor("ff1b", [2 * FMT, D, 128], F32R, kind="ExternalInput").ap()
    dram["ff2"] = nc.dram_tensor("ff2", [DFF, D], BF16, kind="ExternalInput").ap()
    dram["biases"] = nc.dram_tensor("biases", [128, NB], FP32, kind="ExternalInput").ap()
    out_dram = nc.dram_tensor("outT", [D, T], F32R, kind="ExternalOutput").ap()

    scale = float(DH) ** -0.5

    with tile.TileContext(nc) as tc:
        from contextlib import ExitStack
        with ExitStack() as ctx:
            pc = ctx.enter_context(tc.tile_pool(name="const", bufs=1))
            pres = ctx.enter_context(tc.tile_pool(name="res", bufs=5))
            pn = ctx.enter_context(tc.tile_pool(name="n", bufs=5))
            psq = ctx.enter_context(tc.tile_pool(name="sq", bufs=1))
            prow = ctx.enter_context(tc.tile_pool(name="row", bufs=1))
            pbc = ctx.enter_context(tc.tile_pool(name="bc", bufs=2))
            prcb = ctx.enter_context(tc.tile_pool(name="rcb", bufs=2))
            pw = ctx.enter_context(tc.tile_pool(name="w", bufs=7))
            pps = ctx.enter_context(tc.tile_pool(name="ps", bufs=2, space="PSUM"))

            bias_sb = pc.tile([128, NB], FP32, tag="bias")
            nc.sync.dma_start(bias_sb[:], dram["biases"][:])
            invd_f = pc.tile([128, 1], FP32, tag="invdf")
            nc.vector.memset(invd_f[:], 1.0 / D)
            invd = pc.tile([128, 1], F32R, tag="invd")
            nc.vector.tensor_copy(invd[:], invd_f[:])  # fp32r rounding producer
            onesr_f = pc.tile([128, 128], FP32, tag="onesrf")
            nc.vector.memset(onesr_f[:], 1.0)
            onesr = pc.tile([128, 128], F32R, tag="onesr")
            nc.vector.tensor_copy(onesr[:], onesr_f[:])

            def bcol(j):
                return bias_sb[:, j:j + 1]

            def load_w(dname, n_kt, tag, pool, dtype=F32R):
                tiles = []
                for kt in range(n_kt):
                    wt = pool.tile([128, D], dtype, tag=tag, name=f"{dname}_{kt}")
                    nc.sync.dma_start(wt[:], dram[dname][kt * 128:(kt + 1) * 128, :])
                    tiles.append(wt)
                return tiles

            def emit_ln(x_tiles, which, out_tiles):
                """Feature-major LN of 5 [128, T] fp32r tiles.

                Column stats via fp32r ones-matmuls; mean/rstd rows for the
                two query halves are packed at partitions 0/32 so one batched
                DVE reciprocal serves both, and broadcasting across
                partitions is a PE ones-column outer product into PSUM
                (gpsimd partition_broadcast corrupts offset-row sources on
                HW). out_tiles: list that receives the 5 result APs; passing
                x_tiles itself runs the LN in place."""
                in_place = out_tiles is x_tiles
                mup = prow.tile([128, QH], F32R, tag="mup", bufs=2, name=f"mup{which}")
                msqp = prow.tile([128, QH], FP32, tag="msqp", bufs=2, name=f"msqp{which}")
                rstd = prow.tile([128, QH], F32R, tag="rstd", bufs=2, name=f"rstd{which}")
                mu_b = {}
                for hh in range(2):
                    sl = slice(hh * QH, (hh + 1) * QH)
                    r0 = 32 * hh
                    stp = pps.tile([128, 2 * QH], FP32, tag="sps", bufs=2,
                                   name=f"lnps{which}{hh}")
                    sp = stp[:, 0:QH]
                    spq = stp[:, QH:2 * QH]
                    for kt in range(DKT):
                        nc.tensor.matmul(sp[0:1, :], invd[:, 0:1],
                                         x_tiles[kt][:, sl],
                                         start=(kt == 0), stop=(kt == DKT - 1))
                    for kt in range(DKT):
                        sq = psq.tile([128, QH], F32R, tag="sq", name=f"sq{which}{hh}{kt}")
                        nc.scalar.square(sq[:], x_tiles[kt][:, sl])
                        nc.tensor.matmul(spq[0:1, :], invd[:, 0:1], sq[:],
                                         start=(kt == 0), stop=(kt == DKT - 1))
                    nc.vector.tensor_copy(mup[r0:r0 + 1, :], sp[0:1, :])
                    nc.vector.tensor_copy(msqp[r0:r0 + 1, :], spq[0:1, :])
                    mb = pps.tile([128, QH], FP32, tag="avps", bufs=2,
                                  name=f"mub{which}{hh}")
                    nc.tensor.matmul(mb[:, :], onesr[r0:r0 + 1, :],
                                     mup[r0:r0 + 1, :], start=True, stop=True)
                    mu_b[hh] = mb
                    # pass 1: x - mu (frees the mu broadcast PSUM bank early)
                    for kt in range(DKT):
                        if in_place:
                            nt_seg = x_tiles[kt][:, sl]
                        else:
                            if hh == 0:
                                nt = pn.tile([128, T], F32R, tag="n",
                                             name=f"n{which}_{kt}")
                                out_tiles.append(nt)
                            nt_seg = out_tiles[kt][:, sl]
                        nc.vector.tensor_tensor(nt_seg, x_tiles[kt][:, sl],
                                                mu_b[hh][:, :], OP.subtract)
                    # -var = mu^2 - E[x^2] at the packed row
                    nc.vector.tensor_tensor(mup[r0:r0 + 1, :], mup[r0:r0 + 1, :],
                                            mup[r0:r0 + 1, :], OP.mult)
                    nc.vector.tensor_tensor(mup[r0:r0 + 1, :], mup[r0:r0 + 1, :],
                                            msqp[r0:r0 + 1, :], OP.subtract)
                    # rstd = exp(-0.5 * ln(var + eps)); ACT Ln/Exp round trip
                    # measured at 1.1e-5 max rel on HW, and keeps the whole
                    # tail off the (busier) vector engine
                    nc.scalar.activation(msqp[r0:r0 + 1, :], mup[r0:r0 + 1, :],
                                         AF.Ln, scale=-1.0,
                                         bias=bias_sb[0:1, EPS_COL:EPS_COL + 1])
                    nc.scalar.activation(rstd[r0:r0 + 1, :], msqp[r0:r0 + 1, :],
                                         AF.Exp, scale=-0.5)
                for hh in range(2):
                    sl = slice(hh * QH, (hh + 1) * QH)
                    r0 = 32 * hh
                    rb = pps.tile([128, QH], FP32, tag="avps", bufs=2,
                                  name=f"rb{which}{hh}")
                    nc.tensor.matmul(rb[:, :], onesr[r0:r0 + 1, :],
                                     rstd[r0:r0 + 1, :], start=True, stop=True)
                    for kt in range(DKT):
                        nt_seg = (x_tiles[kt] if in_place else out_tiles[kt])[:, sl]
                        nc.vector.tensor_tensor(nt_seg, nt_seg, rb[:, :], OP.mult)
                        if not ln_trivial[which - 1]:
                            nc.scalar.activation(nt_seg, nt_seg, AF.Identity,
                                                 bias=bcol(LN_B[which] + kt),
                                                 scale=bcol(LN_G[which] + kt))
                return out_tiles

            def head_proj(w_tiles, n_tiles, out_tiles, col_off, n_kt, tag):
                """out^T[h][0:80, col_off:col_off+T] = w.T @ n, per-head padded."""
                for h in range(H):
                    for hh in range(2):
                        qp = pps.tile([128, QH], FP32, tag="ps", name=f"hp{tag}{h}{hh}")
                        for kt in range(n_kt):
                            nc.tensor.matmul(
                                qp[0:DH, :],
                                r32(w_tiles[kt][:, h * DH:(h + 1) * DH]),
                                r32(n_tiles[kt][:, hh * QH:(hh + 1) * QH]),
                                start=(kt == 0), stop=(kt == n_kt - 1))
                        nc.vector.tensor_copy(
                            out_tiles[h][0:DH, col_off + hh * QH:col_off + (hh + 1) * QH],
                            qp[0:DH, :])

            def v_proj(n_tiles, vt, n_kt, w_tiles, n_tok, tok_off):
                """token-major V tile, per-head 97-col slots: data cols 0:80,
                ones col at 96 so the AV denominator lands on PSUM partition
                96 (engine APs must start at partition 0/32/64/96)."""
                pad_ap = vt[:, 0:776].rearrange("p (h c) -> p h c", c=97)[:, :, 80:96]
                nc.vector.memset(pad_ap, 0.0)
                ones_ap = vt[:, 0:776].rearrange("p (h c) -> p h c", c=97)[:, :, 96:97]
                nc.vector.memset(ones_ap, 1.0)
                vpp = pps.tile([128, 2 * QH], FP32, tag="sps", bufs=2, name="vpp")
                for half in range(2):
                    vp = vpp[:, half * QH:half * QH + 320]
                    for kt in range(n_kt):
                        nc.tensor.matmul(
                            vp[0:n_tok, :],
                            r32(n_tiles[kt][:, tok_off:tok_off + n_tok]),
                            r32(w_tiles[kt][:, half * 320:(half + 1) * 320]),
                            start=(kt == 0), stop=(kt == n_kt - 1))
                    dst = vt[:, half * 388:half * 388 + 388].rearrange(
                        "p (h c) -> p h c", c=97)[0:n_tok, :, 0:80]
                    src = vp[0:n_tok, :].rearrange("p (h c) -> p h c", c=80)
                    nc.vector.tensor_copy(dst, src)

            def attention(qT_t, kT_t, v_t, n_keytiles, key_dim_last, aT_t, e_pool,
                          recip_on_act=False):
                """S^T -> exp -> AV; attention output is evicted unnormalized
                and the 16 per-(head, q-half) denominators are batched into
                two 32-row-aligned tiles so just two accurate reciprocals run
                (a [1,512] DVE reciprocal costs ~3.3us; 32 of them dominated
                the v1 profile)."""
                den_t = {}
                denr_t = {}

                def dslot(p):
                    return p // 3, 32 * (p % 3)

                def emit_group_normalize(t):
                    """reciprocal of den tile t + normalize its pairs."""
                    dr = prcb.tile([128, QH], F32R, tag="denr", bufs=3,
                                   name=f"denr{t}")
                    if recip_on_act:
                        # 1/x = exp(-ln(x)): ~2e-5 rel, keeps cross-attention
                        # off the vector engine (its PE work is tiny and the
                        # DVE reciprocal would dominate the phase)
                        lt = prcb.tile([128, QH], FP32, tag="denln", bufs=2,
                                       name=f"denln{t}")
                        nc.scalar.activation(lt[:], den_t[t][:], AF.Ln)
                        nc.scalar.activation(dr[:], lt[:], AF.Exp, scale=-1.0)
                    else:
                        with nc.allow_low_precision(reason="fp32r denom rounding"):
                            nc.vector.reciprocal(dr[:], den_t[t][:])
                    denr_t[t] = dr
                    for p in range(3 * t, min(3 * t + 3, n_pairs)):
                        h, hh = p // 2, p % 2
                        _, drow = dslot(p)
                        rcb = pps.tile([128, QH], FP32, tag="ps", bufs=2,
                                       name=f"rcb{h}{hh}")
                        nc.tensor.matmul(
                            rcb[0:DH, :], onesr[drow:drow + 1, 0:DH],
                            dr[drow:drow + 1, :], start=True, stop=True)
                        seg = aT_t[h][0:DH, hh * QH:(hh + 1) * QH]
                        nc.vector.tensor_tensor(seg, seg, rcb[0:DH, :], OP.mult)
                npairs = (n_keytiles + 1) // 2
                n_pairs = 2 * H
                for h in range(H):
                    at = aT_t[h]
                    # rows 80:128 are padding consumed by the padded out-proj;
                    # zero from 64 (SBUF APs must start at partition 0/32/64/96)
                    nc.vector.memset(at[64:128, :], 0.0)
                    for hh in range(2):
                        p = h * 2 + hh
                        avp = pps.tile([128, QH], FP32, tag="avps", bufs=2,
                                       name=f"av{h}{hh}")
                        # two score tiles share one 2-bank PSUM tile so a
                        # single exp covers both (halves the ACT op count);
                        # pipelined one pair ahead of the AV consumers
                        ets = {}
                        for pt in range(npairs + 1):
                            if pt < npairs:
                                kts = [kt for kt in (2 * pt, 2 * pt + 1)
                                       if kt < n_keytiles]
                                spp = pps.tile([128, 2 * QH], FP32, tag="sps",
                                               bufs=2, name=f"s{h}{hh}{pt}")
                                klens = []
                                for j, kt in enumerate(kts):
                                    klen = (key_dim_last
                                            if kt == n_keytiles - 1 else 128)
                                    klens.append(klen)
                                    nc.tensor.matmul(
                                        spp[0:klen, j * QH:(j + 1) * QH],
                                        kT_t[h][0:DH, kt * 128:kt * 128 + klen],
                                        qT_t[h][0:DH, hh * QH:(hh + 1) * QH],
                                        start=True, stop=True)
                                et = e_pool.tile([128, 2 * QH], BF16, tag="E",
                                                 name=f"e{h}{hh}{pt}")
                                if len(kts) == 2 and klens[0] == klens[1]:
                                    nc.scalar.activation(
                                        et[0:klens[0], :], spp[0:klens[0], :],
                                        AF.Exp, scale=scale)
                                else:
                                    for j, kt in enumerate(kts):
                                        nc.scalar.activation(
                                            et[0:klens[j], j * QH:(j + 1) * QH],
                                            spp[0:klens[j], j * QH:(j + 1) * QH],
                                            AF.Exp, scale=scale)
                                ets[pt] = (et, kts, klens)
                            if pt > 0:
                                pet, pkts, pklens = ets.pop(pt - 1)
                                for j, kt in enumerate(pkts):
                                    nc.tensor.matmul(
                                        avp[0:97, :],
                                        v_t[kt][0:pklens[j], h * 97:(h + 1) * 97],
                                        pet[0:pklens[j], j * QH:(j + 1) * QH],
                                        start=(kt == 0), stop=(kt == n_keytiles - 1))
                        # unnormalized evict (frees the PSUM bank) + denom stash
                        nc.vector.tensor_copy(at[0:DH, hh * QH:(hh + 1) * QH],
                                              avp[0:DH, :])
                        dt_i, drow = dslot(p)
                        if dt_i not in den_t:
                            dn = prcb.tile([128, QH], FP32, tag="den", bufs=3,
                                           name=f"den{dt_i}")
                            nc.vector.memset(dn[:], 1.0)
                            den_t[dt_i] = dn
                        nc.vector.tensor_copy(
                            den_t[dt_i][drow:drow + 1, :], avp[96:97, :])
                        if p == 3 * dt_i + 2 or p == n_pairs - 1:
                            emit_group_normalize(dt_i)


            def out_proj(wp_tiles, aT_t, res_t, bias_off):
                """res += aT @ o^T + bias (in-place residual update)."""
                for m in range(DKT):
                    for hh in range(2):
                        op_ = pps.tile([128, QH], FP32, tag="ps", name=f"op{m}{hh}")
                        for kt in range(H):
                            nc.tensor.matmul(
                                op_[:, :],
                                wp_tiles[kt][:, m * 128:(m + 1) * 128],
                                aT_t[kt][:, hh * QH:(hh + 1) * QH],
                                start=(kt == 0), stop=(kt == H - 1))
                        seg = res_t[m][:, hh * QH:(hh + 1) * QH]
                        nc.vector.scalar_tensor_tensor(
                            seg, op_[:, :], bcol(bias_off + m), seg, OP.add, OP.add)

            # residual stream (feature-major, f32)
            res_tiles = []
            for kt in range(DKT):
                rt = pres.tile([128, T], F32R, tag="res", name=f"res_{kt}")
                nc.sync.dma_start(rt[:], dram["hsT_q"][kt * 128:(kt + 1) * 128, :])
                res_tiles.append(rt)

            with ExitStack() as ctx_abcd:
                pqT = ctx_abcd.enter_context(tc.tile_pool(name="qT", bufs=8))
                paT = ctx_abcd.enter_context(tc.tile_pool(name="aT", bufs=8))

                # ---------- phase A: LN1 + QKV projections ----------
                with ExitStack() as ctx_b:
                    pkT = ctx_b.enter_context(tc.tile_pool(name="kT", bufs=8))
                    pV = ctx_b.enter_context(tc.tile_pool(name="V", bufs=16))
                    pE = ctx_b.enter_context(tc.tile_pool(name="E", bufs=4))

                    kT_tiles = [pkT.tile([128, 2 * T], BF16, tag="kT", name=f"kT_{h}")
                                for h in range(H)]
                    v_tiles = [pV.tile([128, 776], BF16, tag="V", name=f"v_{i}")
                               for i in range(2 * TT)]

                    n_q = emit_ln(res_tiles, 1, [])
                    fr0_tiles = []
                    for kt in range(DKT):
                        ft = pn.tile([128, T], F32R, tag="fr", bufs=5,
                                     name=f"fr0_{kt}")
                        nc.sync.dma_start(
                            ft[:], dram["hsT_first"][kt * 128:(kt + 1) * 128, :])
                        fr0_tiles.append(ft)
                    emit_ln(fr0_tiles, 1, fr0_tiles)  # in place, overlaps Q proj
                    q1_sb = load_w("q1", DKT, "w", pw)
                    qT_tiles = [pqT.tile([128, T], BF16, tag="qT", name=f"qT_{h}")
                                for h in range(H)]
                    head_proj(q1_sb, n_q, qT_tiles, 0, DKT, "q")

                    for fi, fr_tiles in enumerate((fr0_tiles, None)):
                        if fr_tiles is None:
                            fr_tiles = []
                            for kt in range(DKT):
                                ft = pn.tile([128, T], F32R, tag="fr", bufs=5,
                                             name=f"fr1_{kt}")
                                nc.sync.dma_start(
                                    ft[:],
                                    dram["hsT_former"][kt * 128:(kt + 1) * 128, :])
                                fr_tiles.append(ft)
                            emit_ln(fr_tiles, 1, fr_tiles)  # in place
                        k1_sb = load_w("k1", DKT, "w", pw)
                        head_proj(k1_sb, fr_tiles, kT_tiles, fi * T, DKT, f"k{fi}")
                        v1_sb = load_w("v1", DKT, "w", pw)
                        for tt in range(TT):
                            v_proj(fr_tiles, v_tiles[fi * TT + tt], DKT, v1_sb,
                                   128, tt * 128)

                    # ---------- phase B: sparse-causal attention ----------
                    aT_tiles = [paT.tile([128, T], BF16, tag="aT", name=f"aT_{h}")
                                for h in range(H)]
                    attention(qT_tiles, kT_tiles, v_tiles, 2 * TT, 128, aT_tiles, pE)

                # ---------- phase C: o1 + residual ----------
                with ExitStack() as ctx_c:
                    pwp = ctx_c.enter_context(tc.tile_pool(name="wp", bufs=8))
                    o1p_sb = load_w("o1p", H, "wp", pwp, dtype=BF16)
                    out_proj(o1p_sb, aT_tiles, res_tiles, OB1)

                # ---------- phase D: cross attention ----------
                with ExitStack() as ctx_d:
                    penc = ctx_d.enter_context(tc.tile_pool(name="enc", bufs=6))
                    pk2 = ctx_d.enter_context(tc.tile_pool(name="k2T", bufs=8))
                    pV2 = ctx_d.enter_context(tc.tile_pool(name="V2", bufs=1))
                    pE2 = ctx_d.enter_context(tc.tile_pool(name="E2", bufs=4))
                    pwp2 = ctx_d.enter_context(tc.tile_pool(name="wp2", bufs=8))

                    n2 = emit_ln(res_tiles, 2, [])
                    q2_sb = load_w("q2", DKT, "w", pw)
                    q2T_tiles = [pqT.tile([128, T], BF16, tag="qT", name=f"q2T_{h}")
                                 for h in range(H)]
                    head_proj(q2_sb, n2, q2T_tiles, 0, DKT, "q2")

                    enc_tiles = []
                    for kt in range(CKT):
                        et_ = penc.tile([128, CTXP], F32R, tag="enc", name=f"enc_{kt}")
                        nc.sync.dma_start(
                            et_[:], dram["encT"][kt * 128:(kt + 1) * 128, :])
                        enc_tiles.append(et_)
                    k2_sb = load_w("k2", CKT, "w", pw)
                    k2T_tiles = [pk2.tile([128, CTXP], BF16, tag="k2T", name=f"k2T_{h}")
                                 for h in range(H)]
                    for h in range(H):
                        kp = pps.tile([128, CTXP], FP32, tag="ps", name=f"k2p{h}")
                        for kt in range(CKT):
                            nc.tensor.matmul(kp[0:DH, :],
                                             r32(k2_sb[kt][:, h * DH:(h + 1) * DH]),
                                             r32(enc_tiles[kt][:]),
                                             start=(kt == 0), stop=(kt == CKT - 1))
                        nc.vector.tensor_copy(k2T_tiles[h][0:DH, :], kp[0:DH, :])
                    v2_sb = load_w("v2", CKT, "w", pw)
                    v2_t = pV2.tile([128, 776], BF16, tag="V2", name="v2t")
                    v_proj(enc_tiles, v2_t, CKT, v2_sb, CTX, 0)

                    a2T_tiles = [paT.tile([128, T], BF16, tag="aT", name=f"a2T_{h}")
                                 for h in range(H)]
                    attention(q2T_tiles, k2T_tiles, [v2_t], 1, CTX, a2T_tiles, pE2,
                              recip_on_act=True)
                    o2p_sb = load_w("o2p", H, "wp2", pwp2, dtype=BF16)
                    out_proj(o2p_sb, a2T_tiles, res_tiles, OB2)

            # ---------- phase E: GEGLU feed-forward ----------
            with ExitStack() as ctx_e:
                pG = ctx_e.enter_context(tc.tile_pool(name="gT", bufs=20))
                pgl = ctx_e.enter_context(tc.tile_pool(name="gl", bufs=3))
                pff2 = ctx_e.enter_context(tc.tile_pool(name="ff2w", bufs=20))

                n3 = emit_ln(res_tiles, 3, [])
                gT_tiles = []
                for mi in range(FMT):
                    fx = pw.tile([128, D], F32R, tag="w", name=f"fx{mi}")
                    fg = pw.tile([128, D], F32R, tag="w", name=f"fg{mi}")
                    fx_dst = fx[:].rearrange("p (k c) -> p k c", c=128)
                    fg_dst = fg[:].rearrange("p (k c) -> p k c", c=128)
                    fx_src = dram["ff1b"][mi].rearrange("(k p) c -> p k c", p=128)
                    fg_src = dram["ff1b"][FMT + mi].rearrange("(k p) c -> p k c", p=128)
                    nc.sync.dma_start(fx_dst, fx_src)
                    nc.sync.dma_start(fg_dst, fg_src)
                    gt = pG.tile([128, T], BF16, tag="gT", name=f"gT_{mi}")
                    gT_tiles.append(gt)
                    for hh in range(2):
                        xgp = pps.tile([128, 2 * QH], FP32, tag="sps", bufs=2,
                                       name=f"xgp{mi}{hh}")
                        xp = xgp[:, 0:QH]
                        gp = xgp[:, QH:2 * QH]
                        for kt in range(DKT):
                            nc.tensor.matmul(
                                xp[:, :], r32(fx[:, kt * 128:(kt + 1) * 128]),
                                r32(n3[kt][:, hh * QH:(hh + 1) * QH]),
                                start=(kt == 0), stop=(kt == DKT - 1))
                        for kt in range(DKT):
                            nc.tensor.matmul(
                                gp[:, :], r32(fg[:, kt * 128:(kt + 1) * 128]),
                                r32(n3[kt][:, hh * QH:(hh + 1) * QH]),
                                start=(kt == 0), stop=(kt == DKT - 1))
                        gl = pgl.tile([128, QH], BF16, tag="gl", name=f"gl{mi}{hh}")
                        nc.scalar.activation(gl[:], gp[:, :], GELU_AF or AF.Gelu,
                                             bias=bcol(FBG + mi), scale=1.0)
                        nc.vector.scalar_tensor_tensor(
                            gt[:, hh * QH:(hh + 1) * QH], xp[:, :], bcol(FBX + mi),
                            gl[:], OP.add, OP.mult)

                ff2_sb = load_w("ff2", FMT, "ff2w", pff2, dtype=BF16)
                for m in range(DKT):
                    for hh in range(2):
                        fp = pps.tile([128, QH], FP32, tag="ps", name=f"fp{m}{hh}")
                        for kt in range(FMT):
                            nc.tensor.matmul(
                                fp[:, :], ff2_sb[kt][:, m * 128:(m + 1) * 128],
                                gT_tiles[kt][:, hh * QH:(hh + 1) * QH],
                                start=(kt == 0), stop=(kt == FMT - 1))
                        seg = res_tiles[m][:, hh * QH:(hh + 1) * QH]
                        nc.vector.scalar_tensor_tensor(
                            seg, fp[:, :], bcol(FB2 + m), seg, OP.add, OP.add)
            for m in range(DKT):
                nc.sync.dma_start(out_dram[m * 128:(m + 1) * 128, :], res_tiles[m][:])

    nc.compile()
    return nc


_PROGRAM_CACHE = {}


def _get_program(ln_trivial):
    key = (tuple(ln_trivial), GELU_AF)
    if key not in _PROGRAM_CACHE:
        _PROGRAM_CACHE[key] = build_program(ln_trivial)
    return _PROGRAM_CACHE[key]


def _pad_heads(w):
    """[640, 640] head rows -> [1024, 640] padded to 128/head."""
    out = np.zeros((H * 128, D), np.float32)
    for h in range(H):
        out[h * 128:h * 128 + DH] = w[h * DH:(h + 1) * DH]
    return out


def _bias_cols(vec, n):
    return np.ascontiguousarray(vec.reshape(n, 128).T)


def kernel(**inputs):
    hs = np.ascontiguousarray(inputs["hidden_states"], np.float32)
    enc = np.ascontiguousarray(inputs["encoder_hidden_states"], np.float32)
    f = int(inputs["video_length"])
    BF = hs.shape[0]
    assert BF == N_CORES and hs.shape[1:] == (T, D)

    ln_trivial = tuple(
        bool(np.all(inputs[f"n{i}_g"] == 1.0) and np.all(inputs[f"n{i}_b"] == 0.0))
        for i in (1, 2, 3))
    nc = _get_program(ln_trivial)

    biases = np.zeros((128, NB), np.float32)
    biases[:, EPS_COL] = LN_EPS
    biases[:, OB1:OB1 + 5] = _bias_cols(inputs["o1_b"].astype(np.float32), 5)
    biases[:, OB2:OB2 + 5] = _bias_cols(inputs["o2_b"].astype(np.float32), 5)
    biases[:, FB2:FB2 + 5] = _bias_cols(inputs["ff2_b"].astype(np.float32), 5)
    ff1_b = inputs["ff1_b"].astype(np.float32)
    biases[:, FBX:FBX + FMT] = _bias_cols(ff1_b[:DFF], FMT)
    biases[:, FBG:FBG + FMT] = _bias_cols(ff1_b[DFF:], FMT)
    for i in (1, 2, 3):
        biases[:, LN_G[i]:LN_G[i] + 5] = _bias_cols(inputs[f"n{i}_g"].astype(np.float32), 5)
        biases[:, LN_B[i]:LN_B[i] + 5] = _bias_cols(inputs[f"n{i}_b"].astype(np.float32), 5)

    ff1 = inputs["ff1"].astype(np.float32)  # [640, 5120]
    ff1b = np.ascontiguousarray(
        ff1.reshape(DKT, 128, 2 * FMT, 128).transpose(2, 0, 1, 3).reshape(2 * FMT, D, 128))

    common = {
        "q1": np.ascontiguousarray(inputs["q1"], np.float32),
        "k1": np.ascontiguousarray(inputs["k1"], np.float32),
        "v1": np.ascontiguousarray(inputs["v1"], np.float32),
        "q2": np.ascontiguousarray(inputs["q2"], np.float32),
        "k2": np.ascontiguousarray(inputs["k2"], np.float32),
        "v2": np.ascontiguousarray(inputs["v2"], np.float32),
        "o1p": _pad_heads(inputs["o1"].astype(np.float32)).astype(ml_dtypes.bfloat16),
        "o2p": _pad_heads(inputs["o2"].astype(np.float32)).astype(ml_dtypes.bfloat16),
        "ff1b": ff1b,
        "ff2": np.ascontiguousarray(inputs["ff2"], np.float32).astype(ml_dtypes.bfloat16),
        "biases": biases,
    }

    hsT = np.ascontiguousarray(hs.transpose(0, 2, 1))      # [BF, 640, 1024]
    encT = np.zeros((BF, CROSS, CTXP), np.float32)         # ctx padded 77 -> 80
    encT[:, :, :CTX] = enc.transpose(0, 2, 1)
    in_maps = []
    for g in range(BF):
        bi, fi = divmod(g, f)
        first = bi * f
        former = bi * f + max(fi - 1, 0)
        in_maps.append({
            **common,
            "hsT_q": hsT[g],
            "hsT_first": hsT[first],
            "hsT_former": hsT[former],
            "encT": encT[g],
        })

    res = run_bass_kernel_spmd(nc, in_maps, core_ids=list(range(N_CORES)),
                               trace=bool(int(os.environ.get("KERNEL_TRACE", "0"))))
    kernel.last_results = res
    out = np.stack([res.results[g]["outT"].T for g in range(BF)])
    return np.ascontiguousarray(out.astype(inputs["hidden_states"].dtype))
